# revision 1
# baseline (speedup 1.0000x reference)
"""Trainium2 Bass kernel for nn_AudioModelM1 (2x Mamba2 + selu + pool + heads).

Sharding: data-parallel over batch — 8 samples -> 8 NeuronCores, one sample per
core, no collectives.  Per-core layout is feature-major (features on SBUF
partitions, tokens on the free dim).  The selective scan uses the chunked
(quadratic-intra / recurrent-inter) Mamba2 formulation with Q=128 token chunks
so all heavy math runs on the TensorEngine.

All ScalarE work uses only {exp, ln, relu, square, copy, identity} so a single
activation table (natural_log_exp_and_others) serves the whole kernel:
  silu(x)    = x * sigmoid(x),  sigmoid = 1/(1+exp(-x)) via Exp + reciprocal
  softplus(x)= ln(1 + exp(x))
  rsqrt(x)   = exp(-0.5 * ln(x))
  selu(x)    = relu(L*x) + min(LA*exp(x), LA) - LA
"""
import sys
sys.path.insert(0, "/opt/trn_rl_repo")

from contextlib import ExitStack

import numpy as np
import ml_dtypes

import concourse.bass as bass
import concourse.tile as tile
from concourse import bacc, mybir
from concourse.bass_utils import run_bass_kernel_spmd

FP32 = mybir.dt.float32
BF16 = mybir.dt.bfloat16
AL = mybir.AluOpType
AF = mybir.ActivationFunctionType

D = 1024
E = 2048
NST = 64
HD = 64
H = 32
DCONV = 4
CCH = E + 2 * NST             # 2176 conv channels (17 tiles)
F = 2 * E + 2 * NST + H       # 4256 in_proj rows
L = 2048
NCORE = 8

BLK = 256
NBLK = L // BLK
Q = 128
QPB = BLK // Q

KT_D = D // 128
MT_F = 34
CT = CCH // 128
ET = E // 128
HP = 4

SELU_L = 1.0507009873554805
SELU_A = 1.6732632423543772
SELU_LA = SELU_L * SELU_A
LN_LA = float(np.log(SELU_LA))

_CACHE = {}


def _bf(x):
    return np.ascontiguousarray(np.asarray(x, np.float32).astype(ml_dtypes.bfloat16))


def _f32(x):
    return np.ascontiguousarray(np.asarray(x, np.float32))


def _prep_layer(w, suf):
    in_w = np.asarray(w["in_proj_w" + suf], np.float32)
    out_w = np.asarray(w["out_proj_w" + suf], np.float32)
    norm_w = np.asarray(w["norm_w" + suf], np.float32)
    conv_w = np.asarray(w["conv_w" + suf], np.float32)
    conv_b = np.asarray(w["conv_b" + suf], np.float32)
    dt_b = np.asarray(w["dt_bias" + suf], np.float32)
    A_log = np.asarray(w["A_log" + suf], np.float32)
    Dp = np.asarray(w["D" + suf], np.float32)

    win = in_w.T.reshape(KT_D, 128, F).transpose(1, 0, 2)
    wo = (out_w * norm_w[None, :]).T
    wout = wo.reshape(ET, 128, D).transpose(1, 0, 2)
    cw = conv_w.reshape(CT, 128, DCONV).transpose(1, 0, 2)
    cb = conv_b.reshape(CT, 128).T
    dx = np.repeat(Dp, HD).reshape(ET, 128).T
    return {
        "win" + suf: _bf(win),
        "wout" + suf: _bf(wout),
        "cw" + suf: _f32(cw),
        "cb" + suf: _f32(cb),
        "dtb" + suf: _f32(dt_b.reshape(H, 1)),
        "A" + suf: _f32(-np.exp(A_log).reshape(H, 1)),
        "dx" + suf: _f32(dx),
    }


def _build():
    nc = bacc.Bacc("TRN2")
    dram = {}

    def din(name, shape, dt):
        dram[name] = nc.dram_tensor(name, list(shape), dt, kind="ExternalInput")
        return dram[name]

    xt = din("xt", (128, KT_D, L), BF16)
    for suf in ("1", "2"):
        din("win" + suf, (128, KT_D, F), BF16)
        din("wout" + suf, (128, ET, D), BF16)
        din("cw" + suf, (128, CT, DCONV), FP32)
        din("cb" + suf, (128, CT), FP32)
        din("dtb" + suf, (H, 1), FP32)
        din("A" + suf, (H, 1), FP32)
        din("dx" + suf, (128, ET), FP32)
    din("maskneg", (128, 128), FP32)
    din("identb", (128, 128), BF16)
    din("identf", (128, 128), FP32)
    din("onesb", (128, 1), BF16)
    din("whead", (128, KT_D, 10), FP32)
    din("bcat", (1, 10), FP32)

    u2 = nc.dram_tensor("u2spill", [128, KT_D, L], BF16)
    out_d = nc.dram_tensor("out", [1, 10], FP32, kind="ExternalOutput")

    with nc.allow_low_precision(reason="bf16 staging is intentional"), \
            tile.TileContext(nc) as tc, ExitStack() as ctx:
        pw = ctx.enter_context(tc.tile_pool(name="weights", bufs=1))
        pconst = ctx.enter_context(tc.tile_pool(name="consts", bufs=1))
        pio = ctx.enter_context(tc.tile_pool(name="io", bufs=2))
        pz = ctx.enter_context(tc.tile_pool(name="zsil", bufs=1))
        pxbc = ctx.enter_context(tc.tile_pool(name="xbcin", bufs=2))
        pxc = ctx.enter_context(tc.tile_pool(name="xconv", bufs=1))
        pg = ctx.enter_context(tc.tile_pool(name="gate", bufs=2))
        psc = ctx.enter_context(tc.tile_pool(name="scan", bufs=3))
        pxt = ctx.enter_context(tc.tile_pool(name="xtok", bufs=1))
        pcm = ctx.enter_context(tc.tile_pool(name="chunkmeta", bufs=1))
        pb1 = ctx.enter_context(tc.tile_pool(name="bcq1", bufs=2))
        pb2 = ctx.enter_context(tc.tile_pool(name="bcq2", bufs=2))
        psm = ctx.enter_context(tc.tile_pool(name="small", bufs=2))
        pstate = ctx.enter_context(tc.tile_pool(name="state", bufs=1))

        ps_mm = ctx.enter_context(tc.tile_pool(name="psmm", bufs=3, space="PSUM"))
        ps_tr = ctx.enter_context(tc.tile_pool(name="pstr", bufs=1, space="PSUM"))
        ps_g0 = ctx.enter_context(tc.tile_pool(name="psg0", bufs=1, space="PSUM"))
        ps_yp = ctx.enter_context(tc.tile_pool(name="psyp", bufs=2, space="PSUM"))
        ps_sp = ctx.enter_context(tc.tile_pool(name="pssp", bufs=1, space="PSUM"))

        w_in = pw.tile([128, KT_D, F], BF16)
        w_out = pw.tile([128, ET, D], BF16)
        cw = pw.tile([128, CT, DCONV], FP32)
        cb = pw.tile([128, CT], FP32)
        dtb = pw.tile([H, 1], FP32)
        Atile = pw.tile([H, 1], FP32)
        dxt = pw.tile([128, ET], FP32)

        maskneg = pconst.tile([128, 128], FP32)
        identb = pconst.tile([128, 128], BF16)
        identf = pconst.tile([3 * H, 3 * H], FP32)
        onesb = pconst.tile([128, 1], BF16)
        whead = pconst.tile([128, KT_D, 10], FP32)
        bcat = pconst.tile([1, 10], FP32)
        zeros32 = pconst.tile([H, Q], FP32)
        eps_t = pconst.tile([1, 1], FP32)
        ones32 = pconst.tile([H, 1], FP32)
        lnla_t = pconst.tile([128, 1], FP32)

        S = pstate.tile([NST, H * HD], BF16)   # [n, (h,p)] all heads at base partition 0
        pacc = pstate.tile([128, KT_D], FP32)

        for t, name in ((maskneg, "maskneg"), (identb, "identb"),
                        (onesb, "onesb"), (whead, "whead"), (bcat, "bcat")):
            nc.sync.dma_start(t[:], dram[name][:])
        nc.sync.dma_start(identf[:], dram["identf"][0:3 * H, 0:3 * H])
        nc.vector.memset(zeros32[:], 0.0)
        nc.vector.memset(pacc[:], 0.0)
        nc.vector.memset(eps_t[:], 1e-5)
        nc.vector.memset(ones32[:], 1.0)
        nc.vector.memset(lnla_t[:], LN_LA)

        for layer in (0, 1):
            suf = "12"[layer]
            for t, name in ((w_in, "win"), (w_out, "wout"), (cw, "cw"), (cb, "cb"),
                            (dtb, "dtb"), (Atile, "A"), (dxt, "dx")):
                nc.sync.dma_start(t[:], dram[name + suf][:])
            nc.vector.memset(S[:], 0.0)

            prev_xbc = None
            for b in range(NBLK):
                tsl = slice(b * BLK, (b + 1) * BLK)
                u_t = pio.tile([128, KT_D, BLK], BF16)
                src = xt if layer == 0 else u2
                nc.sync.dma_start(u_t[:], src[:, :, tsl])

                # ---- in_proj
                sz = pz.tile([128, ET, BLK], BF16)          # silu(z)
                xbc = pxbc.tile([128, CT, BLK + HP], BF16)  # conv input (padded)
                dt_sb = pcm.tile([H, BLK], FP32, tag="dt")
                if b == 0:
                    nc.vector.memset(xbc[:, :, 0:HP], 0.0)
                else:
                    nc.scalar.copy(xbc[:, :, 1:HP],
                                   prev_xbc[:, :, BLK + 1:BLK + HP])
                for mt in range(MT_F):
                    mm = 128 if mt < 33 else 32
                    pmm = ps_mm.tile([mm, BLK], FP32, tag="mm")
                    for kt in range(KT_D):
                        nc.tensor.matmul(
                            pmm[:], w_in[:, kt, mt * 128:mt * 128 + mm],
                            u_t[:, kt, :], start=(kt == 0), stop=(kt == KT_D - 1))
                    if mt < ET:
                        # silu(z) = z * sigmoid(z)
                        ez = psm.tile([128, BLK], FP32, tag="ezu")
                        nc.scalar.activation(ez[:], pmm[:], AF.Exp, scale=-1.0)
                        nc.vector.tensor_scalar(ez[:], ez[:], 1.0, None, op0=AL.add)
                        nc.vector.reciprocal(ez[:], ez[:])
                        nc.vector.tensor_mul(sz[:, mt, :], pmm[:], ez[:])
                    elif mt < 33:
                        nc.vector.tensor_copy(xbc[:, mt - ET, HP:HP + BLK], pmm[:])
                    else:
                        # dt = softplus(raw + dt_bias) = ln(1 + exp(raw + b))
                        nc.scalar.activation(pmm[:], pmm[:], AF.Exp, bias=dtb[:])
                        nc.scalar.activation(dt_sb[:], pmm[:], AF.Ln,
                                             bias=ones32[:])
                prev_xbc = xbc

                # ---- causal depthwise conv (+bias) then silu
                xc = pxc.tile([128, ET, BLK], BF16)
                bc = psm.tile([128, BLK], BF16, tag="bc")
                for cp in range(9):
                    n_in = 2 if cp < 8 else 1
                    cv = pcm.tile([128, 2 * BLK], FP32, tag="cvp")
                    for i in range(n_in):
                        ct = 2 * cp + i
                        eng = nc.vector
                        cvs = cv[:, i * BLK:(i + 1) * BLK]
                        eng.tensor_scalar(cvs, xbc[:, ct, 1:1 + BLK],
                                          cw[:, ct, 0:1], cb[:, ct:ct + 1],
                                          op0=AL.mult, op1=AL.add)
                        for k in range(1, DCONV):
                            eng.scalar_tensor_tensor(
                                cvs, xbc[:, ct, 1 + k:1 + k + BLK],
                                cw[:, ct, k:k + 1], cvs, op0=AL.mult, op1=AL.add)
                    w = n_in * BLK
                    ec = pcm.tile([128, 2 * BLK], BF16, tag="ecp")
                    nc.scalar.activation(ec[:, 0:w], cv[:, 0:w], AF.Exp, scale=-1.0)
                    nc.vector.tensor_scalar(ec[:, 0:w], ec[:, 0:w], 1.0, None,
                                            op0=AL.add)
                    nc.vector.reciprocal(ec[:, 0:w], ec[:, 0:w])
                    for i in range(n_in):
                        ct = 2 * cp + i
                        dst = xc[:, ct, :] if ct < ET else bc[:]
                        nc.vector.tensor_mul(dst, cv[:, i * BLK:(i + 1) * BLK],
                                             ec[:, i * BLK:(i + 1) * BLK])
                ctc = psm.tile([NST, BLK], BF16, tag="ctc")
                nc.sync.dma_start(ctc[:], bc[NST:128, :])

                alog = pcm.tile([H, BLK], FP32, tag="alog")
                nc.vector.tensor_scalar(alog[:], dt_sb[:], Atile[:], None,
                                        op0=AL.mult)

                scaleb = psm.tile([128, BLK], BF16, tag="scaleb")
                g_sb = pg.tile([128, ET, BLK], BF16)

                for qi in range(QPB):
                    qsl = slice(qi * Q, (qi + 1) * Q)
                    cum = pcm.tile([H, Q], FP32, tag="cum")
                    nc.vector.tensor_tensor_scan(cum[:], alog[:, qsl], zeros32[:],
                                                 0.0, op0=AL.add, op1=AL.add)
                    dst8 = pcm.tile([H, Q], FP32, tag="dst8")
                    nc.scalar.activation(dst8[:], cum[:], AF.Exp, scale=-1.0,
                                         bias=cum[:, Q - 1:Q])
                    sbt = pcm.tile([H, Q], FP32, tag="sbt")
                    nc.vector.tensor_mul(sbt[:], dst8[:], dt_sb[:, qsl])

                    stk = pcm.tile([3 * H, Q], FP32, tag="stk")
                    nc.scalar.copy(stk[0:H, :], cum[:])
                    nc.sync.dma_start(stk[H:2 * H, :], sbt[:])
                    nc.sync.dma_start(stk[2 * H:3 * H, :], dt_sb[:, qsl])
                    ptr = ps_tr.tile([Q, 3 * H], FP32, tag="tr")
                    nc.tensor.transpose(ptr[:], stk[:], identf[0:3 * H, 0:3 * H])
                    ctall = pcm.tile([Q, 3 * H], FP32, tag="ctall")
                    nc.scalar.copy(ctall[:], ptr[:])
                    negcum = pcm.tile([Q, H], FP32, tag="negcum")
                    nc.vector.tensor_scalar(negcum[:], ctall[:, 0:H], -1.0, None,
                                            op0=AL.mult)

                    ptb = ps_tr.tile([Q, NST], BF16, tag="tr")
                    nc.tensor.transpose(ptb[:], bc[0:NST, qsl],
                                        identb[0:NST, 0:NST])
                    btok = pcm.tile([Q, NST], BF16, tag="btok")
                    nc.scalar.copy(btok[:], ptb[:])

                    g0 = ps_g0.tile([Q, Q], FP32)
                    nc.tensor.matmul(g0[:], bc[0:NST, qsl], ctc[:, qsl])

                    xtok = pxt.tile([Q, E], BF16, tag="xtok")
                    for ft in range(ET):
                        ptx = ps_tr.tile([Q, 128], BF16, tag="tr")
                        nc.tensor.transpose(ptx[:], xc[:, ft, qsl], identb[:])
                        nc.scalar.copy(xtok[:, ft * 128:(ft + 1) * 128], ptx[:])

                    g0sb = pcm.tile([Q, Q], BF16, tag="g0sb")
                    nc.vector.tensor_copy(g0sb[:], g0[:])
                    for hg in range(H // 4):
                        h0 = hg * 4
                        stg = psc.tile([1, 4 * Q], FP32, tag="stg")
                        nc.sync.dma_start(stg[:], cum[h0:h0 + 4, :])
                        bcq4 = pb2.tile([Q, 4 * Q], FP32, tag="bcq")
                        nc.gpsimd.partition_broadcast(bcq4[:], stg[:])
                        bce4 = pb1.tile([Q, 4 * Q], FP32, tag="bce")
                        nc.scalar.activation(bce4[:], bcq4[:], AF.Exp)
                        for k in range(4):
                            h = h0 + k
                            ft, ro = h // 2, (h % 2) * 64
                            csl = slice(h * HD, (h + 1) * HD)
                            ksl = slice(k * Q, (k + 1) * Q)
                            nc.gpsimd.tensor_add(bcq4[:, ksl], bcq4[:, ksl],
                                                 maskneg[:])
                            lt = psc.tile([Q, Q], FP32, tag="lt")
                            nc.scalar.activation(lt[:], bcq4[:, ksl], AF.Exp,
                                                 bias=negcum[:, h:h + 1])
                            mt_t = psc.tile([Q, Q], BF16, tag="mt")
                            nc.vector.scalar_tensor_tensor(
                                mt_t[:], g0sb[:],
                                ctall[:, 2 * H + h:2 * H + h + 1],
                                lt[:], op0=AL.mult, op1=AL.mult)
                            cpos = psc.tile([NST, Q], BF16, tag="cpos")
                            nc.vector.tensor_mul(cpos[:], ctc[:, qsl],
                                                 bce4[0:NST, ksl])
                            bh = psc.tile([Q, NST], BF16, tag="bh")
                            nc.vector.tensor_scalar(bh[:], btok[:],
                                                    ctall[:, H + h:H + h + 1],
                                                    None, op0=AL.mult)
                            sb16 = psc.tile([NST, HD], BF16, tag="sb16")
                            nc.vector.tensor_copy(sb16[:], S[:, csl])

                            yp = ps_yp.tile([HD, Q], FP32, tag="yp")
                            nc.tensor.matmul(yp[:], xtok[:, csl], mt_t[:],
                                             start=True, stop=False)
                            nc.tensor.matmul(yp[:], sb16[:], cpos[:],
                                             start=False, stop=True)
                            sp = ps_sp.tile([NST, HD], FP32, tag="sp")
                            nc.tensor.matmul(sp[:], bh[:], xtok[:, csl])
                            nc.vector.scalar_tensor_tensor(
                                S[:, csl], S[:, csl],
                                bce4[0:NST, (k + 1) * Q - 1:(k + 1) * Q], sp[:],
                                op0=AL.mult, op1=AL.add)
                            nc.vector.scalar_tensor_tensor(
                                g_sb[ro:ro + 64, ft, qsl],
                                xc[ro:ro + 64, ft, qsl],
                                dxt[ro:ro + 64, ft:ft + 1], yp[:],
                                op0=AL.mult, op1=AL.add)

                    # ---- gating + sum of squares
                    ssq = ps_sp.tile([1, Q], FP32, tag="sp")
                    for ft in range(ET):
                        nc.vector.tensor_mul(g_sb[:, ft, qsl], g_sb[:, ft, qsl],
                                             sz[:, ft, qsl])
                        g2 = psc.tile([128, Q], BF16, tag="mt")
                        nc.scalar.activation(g2[:], g_sb[:, ft, qsl], AF.Square)
                        nc.tensor.matmul(ssq[:], onesb[:], g2[:],
                                         start=(ft == 0), stop=(ft == ET - 1))
                    # rsqrt(mean + eps) = exp(-0.5 * ln(ssq/E + eps))
                    nc.scalar.activation(ssq[:], ssq[:], AF.Ln,
                                         scale=1.0 / E, bias=eps_t[:])
                    rs = psm.tile([1, Q], BF16, tag="rs")
                    nc.scalar.activation(rs[:], ssq[:], AF.Exp, scale=-0.5)
                    nc.gpsimd.partition_broadcast(scaleb[:, qsl], rs[:])

                # ---- out_proj + rmsnorm scale + selu
                for mt in range(ET // 2):
                    ho = ps_mm.tile([128, BLK], FP32, tag="mm")
                    for kt in range(ET):
                        nc.tensor.matmul(ho[:], w_out[:, kt, mt * 128:(mt + 1) * 128],
                                         g_sb[:, kt, :],
                                         start=(kt == 0), stop=(kt == ET - 1))
                    t1 = psm.tile([128, BLK], FP32, tag="t1")
                    nc.vector.tensor_mul(t1[:], ho[:], scaleb[:])
                    rl = psm.tile([128, BLK], FP32, tag="rl")
                    nc.scalar.activation(rl[:], t1[:], AF.Relu, scale=SELU_L)
                    ex = psm.tile([128, BLK], FP32, tag="ex")
                    nc.scalar.activation(ex[:], t1[:], AF.Exp, bias=lnla_t[:])
                    nc.vector.tensor_scalar_min(ex[:], ex[:], SELU_LA)
                    u2t = psm.tile([128, BLK], BF16, tag="ezu")
                    nc.vector.scalar_tensor_tensor(u2t[:], ex[:], -SELU_LA, rl[:],
                                                   op0=AL.add, op1=AL.add)
                    if layer == 0:
                        nc.sync.dma_start(u2[:, mt, tsl], u2t[:])
                    else:
                        red = psm.tile([128, 1], FP32, tag="red")
                        nc.vector.tensor_reduce(red[:], u2t[:],
                                                axis=mybir.AxisListType.X,
                                                op=AL.add)
                        nc.vector.tensor_add(pacc[:, mt:mt + 1],
                                             pacc[:, mt:mt + 1], red[:])

        pooled = psm.tile([128, KT_D], FP32, tag="pooled")
        nc.vector.tensor_scalar(pooled[:], pacc[:], 1.0 / L, None, op0=AL.mult)
        ph = ps_sp.tile([1, 10], FP32, tag="sp")
        for kt in range(KT_D):
            nc.tensor.matmul(ph[:], pooled[:, kt:kt + 1], whead[:, kt, :],
                             start=(kt == 0), stop=(kt == KT_D - 1))
        ot = psm.tile([1, 10], FP32, tag="ot")
        nc.vector.tensor_add(ot[:], ph[:], bcat[:])
        nc.sync.dma_start(out_d[:], ot[:])

    nc.compile()
    return nc


def _host_inputs(inputs):
    m = {}
    m.update(_prep_layer(inputs, "1"))
    m.update(_prep_layer(inputs, "2"))
    j = np.arange(128)
    m["maskneg"] = _f32(np.where(j[None, :] >= j[:, None], 0.0, -1e30))
    m["identb"] = _bf(np.eye(128))
    m["identf"] = _f32(np.eye(128))
    m["onesb"] = _bf(np.ones((128, 1)))
    wcat = np.concatenate([np.asarray(inputs["w_emo"], np.float32),
                           np.asarray(inputs["w_sen"], np.float32)], 0)
    m["whead"] = _f32(wcat.T.reshape(KT_D, 128, 10).transpose(1, 0, 2))
    m["bcat"] = _f32(np.concatenate([inputs["b_emo"], inputs["b_sen"]])
                     .reshape(1, 10))
    return m


def kernel(**inputs) -> np.ndarray:
    if "nc" not in _CACHE:
        _CACHE["nc"] = _build()
    nc = _CACHE["nc"]

    x = np.asarray(inputs["x"], np.float32)
    shared = _host_inputs(inputs)
    in_maps = []
    for s in range(NCORE):
        m = dict(shared)
        xts = x[s].T.reshape(KT_D, 128, L).transpose(1, 0, 2)
        m["xt"] = _bf(xts)
        in_maps.append(m)

    res = run_bass_kernel_spmd(nc, in_maps, core_ids=list(range(NCORE)))
    out = np.concatenate([r["out"] for r in res.results], 0)
    return out.astype(np.float32)



# revision 33
# speedup vs baseline: 1.3216x; 1.3216x over previous
"""Trainium2 Bass kernel for nn_AudioModelM1 (2x Mamba2 + selu + pool + heads).

Sharding: data-parallel over batch — 8 samples -> 8 NeuronCores, one sample per
core, no collectives.  Per-core layout is feature-major (features on SBUF
partitions, tokens on the free dim).  The selective scan uses the chunked
(quadratic-intra / recurrent-inter) Mamba2 formulation with Q=128 token chunks
so all heavy math runs on the TensorEngine.

Engine-balance notes (CoreSim cost model):
  - dt (softplus) is hoisted to a per-layer preamble and the RMSNorm scale +
    SELU are deferred to a per-layer epilogue so Ln never interleaves with the
    Exp/Tanh activations inside the block loop (act-table reloads are 1.3us).
  - silu(x) = x * (0.5*tanh(x/2) + 0.5): Tanh lives in the same activation
    table as Exp, so no table switches and no DVE reciprocal.
  - Small elementwise work in the scan inner loop runs on the Pool engine
    (flat 0.833 ns/elem, no access penalty); DVE ops keep all tensor operands
    packed bf16 in SBUF to hit the 2x/4x DVE perf modes.
  - Decay matrices are built per 4-head group: Pool stt folds (-cum_s + mask)
    so the Act exp runs batched over [Q, 4Q].
"""
import sys
sys.path.insert(0, "/opt/trn_rl_repo")

from contextlib import ExitStack

import numpy as np
import ml_dtypes

import concourse.bass as bass
import concourse.tile as tile
from concourse import bacc, mybir
from concourse.bass_utils import run_bass_kernel_spmd

FP32 = mybir.dt.float32
BF16 = mybir.dt.bfloat16
AL = mybir.AluOpType
AF = mybir.ActivationFunctionType

D = 1024
E = 2048
NST = 64
HD = 64
H = 32
DCONV = 4
CCH = E + 2 * NST             # 2176 conv channels (17 tiles)
F = 2 * E + 2 * NST + H       # 4256 in_proj rows
L = 2048
NCORE = 8

BLK = 256
NBLK = L // BLK
Q = 128
QPB = BLK // Q

KT_D = D // 128
MT_F = 34
CT = CCH // 128
ET = E // 128
HP = 4

SELU_L = 1.0507009873554805
SELU_A = 1.6732632423543772
SELU_LA = SELU_L * SELU_A
LN_LA = float(np.log(SELU_LA))
EPS = 1e-5

_CACHE = {}


def _bf(x):
    return np.ascontiguousarray(np.asarray(x, np.float32).astype(ml_dtypes.bfloat16))


def _f32(x):
    return np.ascontiguousarray(np.asarray(x, np.float32))


def _prep_layer(w, suf):
    in_w = np.asarray(w["in_proj_w" + suf], np.float32)
    out_w = np.asarray(w["out_proj_w" + suf], np.float32)
    norm_w = np.asarray(w["norm_w" + suf], np.float32)
    conv_w = np.asarray(w["conv_w" + suf], np.float32)
    conv_b = np.asarray(w["conv_b" + suf], np.float32)
    dt_b = np.asarray(w["dt_bias" + suf], np.float32)
    A_log = np.asarray(w["A_log" + suf], np.float32)
    Dp = np.asarray(w["D" + suf], np.float32)

    win = in_w.T.reshape(KT_D, 128, F).transpose(1, 0, 2)
    wo = (out_w * norm_w[None, :]).T
    wout = wo.reshape(ET, 128, D).transpose(1, 0, 2)
    cw = conv_w.reshape(CT, 128, DCONV).transpose(1, 0, 2)
    cb = conv_b.reshape(CT, 128).T
    dx = np.repeat(Dp, HD).reshape(ET, 128).T
    return {
        "win" + suf: _bf(win),
        "wout" + suf: _bf(wout),
        "cw" + suf: _f32(cw),
        "cb" + suf: _f32(cb),
        "dtb" + suf: _f32(dt_b.reshape(H, 1)),
        "A" + suf: _f32(-np.exp(A_log).reshape(H, 1)),
        "dx" + suf: _f32(dx),
    }


def _build():
    nc = bacc.Bacc("TRN2")
    dram = {}

    def din(name, shape, dt):
        dram[name] = nc.dram_tensor(name, list(shape), dt, kind="ExternalInput")
        return dram[name]

    xt = din("xt", (128, KT_D, L), BF16)
    for suf in ("1", "2"):
        din("win" + suf, (128, KT_D, F), BF16)
        din("wout" + suf, (128, ET, D), BF16)
        din("cw" + suf, (128, CT, DCONV), FP32)
        din("cb" + suf, (128, CT), FP32)
        din("dtb" + suf, (H, 1), FP32)
        din("A" + suf, (H, 1), FP32)
        din("dx" + suf, (128, ET), FP32)
    din("mask8", (128, 8 * Q), BF16)
    din("identb", (128, 128), BF16)
    din("identf", (128, 128), FP32)
    din("onesb", (128, 1), BF16)
    din("whead", (128, KT_D, 10), FP32)
    din("bcat", (1, 10), FP32)

    u2 = nc.dram_tensor("u2spill", [128, KT_D, L], BF16)
    ho_d = nc.dram_tensor("hospill", [128, KT_D, L], BF16)
    out_d = nc.dram_tensor("out", [1, 10], FP32, kind="ExternalOutput")

    with nc.allow_low_precision(reason="bf16 staging is intentional"), \
            tile.TileContext(nc) as tc, ExitStack() as ctx:
        pw = ctx.enter_context(tc.tile_pool(name="weights", bufs=1))
        pconst = ctx.enter_context(tc.tile_pool(name="consts", bufs=1))
        pio = ctx.enter_context(tc.tile_pool(name="io", bufs=2))
        pz = ctx.enter_context(tc.tile_pool(name="zsil", bufs=1))
        pxbc = ctx.enter_context(tc.tile_pool(name="xbcin", bufs=1))
        pxc = ctx.enter_context(tc.tile_pool(name="xconv", bufs=1))
        pg = ctx.enter_context(tc.tile_pool(name="gate", bufs=1))
        psc = ctx.enter_context(tc.tile_pool(name="scan", bufs=2))
        pxt = ctx.enter_context(tc.tile_pool(name="xtok", bufs=1))
        pcm = ctx.enter_context(tc.tile_pool(name="chunkmeta", bufs=1))
        pb1 = ctx.enter_context(tc.tile_pool(name="bcq1", bufs=2))
        pb2 = ctx.enter_context(tc.tile_pool(name="bcq2", bufs=2))
        psm = ctx.enter_context(tc.tile_pool(name="small", bufs=2))
        pstate = ctx.enter_context(tc.tile_pool(name="state", bufs=1))
        pdt = ctx.enter_context(tc.tile_pool(name="dtpre", bufs=1))
        pstg = ctx.enter_context(tc.tile_pool(name="stgp", bufs=1))
        pcv1 = ctx.enter_context(tc.tile_pool(name="cv1", bufs=1))

        ps_mm = ctx.enter_context(tc.tile_pool(name="psmm", bufs=2, space="PSUM"))
        ps_tr = ctx.enter_context(tc.tile_pool(name="pstr", bufs=2, space="PSUM"))
        ps_yp = ctx.enter_context(tc.tile_pool(name="psyp", bufs=2, space="PSUM"))
        ps_sp = ctx.enter_context(tc.tile_pool(name="pssp", bufs=2, space="PSUM"))

        w_in = pw.tile([128, KT_D, F], BF16)
        w_out = pw.tile([128, ET, D], BF16)
        cw = pw.tile([128, CT, DCONV], FP32)
        cb = pw.tile([128, CT], FP32)
        dtb = pw.tile([H, 1], FP32)
        Atile = pw.tile([H, 1], FP32)
        dxt = pw.tile([128, ET], FP32)

        mask8 = pconst.tile([128, 8 * Q], BF16)
        identb = pconst.tile([128, 128], BF16)
        identf = pconst.tile([3 * H, 3 * H], FP32)
        onesb = pconst.tile([128, 1], BF16)
        whead = pconst.tile([128, KT_D, 10], FP32)
        bcat = pconst.tile([1, 10], FP32)
        zeros32 = pconst.tile([H, Q], FP32)
        ones32 = pconst.tile([H, 1], FP32)
        lnla_t = pconst.tile([128, 1], FP32)

        S = pstate.tile([NST, H * HD], BF16)   # [n, (h,p)] heads at base part 0
        pacc = pstate.tile([128, ET], FP32)

        # per-layer hoisted dt: raw -> exp -> (ln in place) -> dt; then alog
        dtf = pdt.tile([H, L], FP32)     # exp(raw+b) -> dt (f32) -> alog=dt*A
        dt16 = pdt.tile([H, L], BF16)    # dt snapshot in bf16
        dttok = pdt.tile([Q, NBLK * QPB * H], FP32)  # token-major dt columns
        mall = pdt.tile([1, L], BF16)    # ssq/E+eps -> ln -> rmsnorm scale
        tailt = pdt.tile([128, CT, HP - 1], BF16)  # conv tail carry

        for t, name in ((mask8, "mask8"), (identb, "identb"),
                        (onesb, "onesb"), (whead, "whead"), (bcat, "bcat")):
            nc.sync.dma_start(t[:], dram[name][:])
        nc.sync.dma_start(identf[:], dram["identf"][0:3 * H, 0:3 * H])
        nc.vector.memset(zeros32[:], 0.0)
        nc.vector.memset(pacc[:], 0.0)
        nc.vector.memset(ones32[:], 1.0)
        nc.vector.memset(lnla_t[:], LN_LA)

        for layer in (0, 1):
            suf = "12"[layer]
            for t, name in ((w_in, "win"), (w_out, "wout"), (cw, "cw"), (cb, "cb"),
                            (dtb, "dtb"), (Atile, "A"), (dxt, "dx")):
                nc.sync.dma_start(t[:], dram[name + suf][:])
            nc.vector.memset(S[:], 0.0)

            src = xt if layer == 0 else u2

            # ---- dt preamble: dt_raw for the whole layer, one softplus ----
            for b in range(NBLK):
                tsl = slice(b * BLK, (b + 1) * BLK)
                u_t = pio.tile([128, KT_D, BLK], BF16, tag="ut")
                nc.sync.dma_start(u_t[:], src[:, :, tsl])
                pmmd = ps_mm.tile([H, BLK], FP32, tag="mm")
                for kt in range(KT_D):
                    nc.tensor.matmul(pmmd[:], w_in[:, kt, F - H:F],
                                     u_t[:, kt, :], start=(kt == 0),
                                     stop=(kt == KT_D - 1))
                nc.scalar.activation(dtf[:, tsl], pmmd[:], AF.Exp, bias=dtb[:])
            # softplus ln over the whole layer at once, then dt16 + alog
            nc.scalar.activation(dtf[:], dtf[:], AF.Ln, bias=ones32[:])
            nc.vector.tensor_copy(dt16[:], dtf[:])
            nc.vector.tensor_scalar(dtf[:], dtf[:], Atile[:], None, op0=AL.mult)
            # dtf now holds alog = dt * A
            # token-major dt columns for the whole layer: dttok[:, c*H+h]
            for c in range(NBLK * QPB):
                ptd = ps_tr.tile([Q, H], BF16, tag="tr")
                nc.tensor.transpose(ptd[:], dt16[:, c * Q:(c + 1) * Q],
                                    identb[0:H, 0:H])
                nc.vector.tensor_copy(dttok[:, c * H:(c + 1) * H], ptd[:])

            xbc = pxbc.tile([128, CT, BLK + HP], BF16)
            nc.vector.memset(xbc[:, :, 0:HP], 0.0)

            for b in range(NBLK):
                tsl = slice(b * BLK, (b + 1) * BLK)
                u_t = pio.tile([128, KT_D, BLK], BF16, tag="ut")
                nc.sync.dma_start(u_t[:], src[:, :, tsl])

                sz = pz.tile([128, ET, BLK], BF16)
                if b > 0:
                    nc.vector.tensor_copy(tailt[:],
                                          xbc[:, :, BLK + 1:BLK + HP])
                # ---- in_proj (z tiles 0..15 -> silu; xbc tiles 16..32)
                for mt in range(33):
                    pmm = ps_mm.tile([128, BLK], FP32, tag="mm")
                    for kt in range(KT_D):
                        nc.tensor.matmul(
                            pmm[:], w_in[:, kt, mt * 128:(mt + 1) * 128],
                            u_t[:, kt, :], start=(kt == 0), stop=(kt == KT_D - 1))
                    if mt < ET:
                        # silu(z) = z * (0.5*tanh(z/2) + 0.5)
                        th = psm.tile([128, BLK], BF16, tag="th")
                        nc.scalar.activation(th[:], pmm[:], AF.Tanh, scale=0.5)
                        zb = psm.tile([128, BLK], BF16, tag="zb")
                        nc.scalar.copy(zb[:], pmm[:])
                        nc.vector.tensor_scalar(th[:], th[:], 0.5, 0.5,
                                                op0=AL.mult, op1=AL.add)
                        nc.gpsimd.tensor_mul(sz[:, mt, :], th[:], zb[:])
                    else:
                        nc.vector.tensor_copy(xbc[:, mt - ET, HP:HP + BLK],
                                              pmm[:])
                if b > 0:
                    nc.vector.tensor_copy(xbc[:, :, 1:HP], tailt[:])

                # ---- causal depthwise conv (+bias): 4 DVE tensor-scalar
                # products (4x perf mode) + 3 Pool tensor-tensor adds
                cv = pxc.tile([128, CT, BLK], BF16)
                for ct in range(CT):
                    cvs = cv[:, ct, :]
                    cvt = pcv1.tile([128, 3, BLK], BF16, tag="cvt")
                    nc.vector.tensor_scalar(cvs, xbc[:, ct, 1:1 + BLK],
                                            cw[:, ct, 0:1], cb[:, ct:ct + 1],
                                            op0=AL.mult, op1=AL.add)
                    for k in range(1, DCONV):
                        nc.vector.tensor_scalar(cvt[:, k - 1, :],
                                                xbc[:, ct, 1 + k:1 + k + BLK],
                                                cw[:, ct, k:k + 1], None,
                                                op0=AL.mult)
                    nc.gpsimd.tensor_add(cvt[:, 0, :], cvt[:, 0, :],
                                         cvt[:, 1, :])
                    nc.gpsimd.tensor_add(cvs, cvs, cvt[:, 2, :])
                    nc.gpsimd.tensor_add(cvs, cvs, cvt[:, 0, :])
                # silu over channel quads, in place (cv becomes conv output)
                for cp in ((0, 4), (4, 4), (8, 4), (12, 4), (16, 1)):
                    c0, n_in = cp
                    w = n_in * BLK
                    cvp = cv[:, c0:c0 + n_in, :]
                    th2 = pcm.tile([128, 4 * BLK], BF16, tag="th2")
                    nc.scalar.activation(th2[:, 0:w], cvp, AF.Tanh, scale=0.5)
                    nc.vector.tensor_scalar(th2[:, 0:w], th2[:, 0:w], 0.5, 0.5,
                                            op0=AL.mult, op1=AL.add)
                    nc.gpsimd.tensor_tensor(cvp, th2[:, 0:w], cvp, op=AL.mult)
                # cv[:, 0:16] = x (silu'd), cv[:, 16] = B (parts 0:64) | C
                ctc = psm.tile([NST, BLK], BF16, tag="ctc")
                nc.sync.dma_start(ctc[:], cv[NST:128, 16, :])

                g_sb = pg.tile([128, ET, BLK], BF16)

                for qi in range(QPB):
                    qsl = slice(qi * Q, (qi + 1) * Q)
                    gsl = slice(b * BLK + qi * Q, b * BLK + (qi + 1) * Q)
                    cidx = b * QPB + qi
                    cum = pcm.tile([H, Q], FP32, tag="cum")
                    nc.vector.tensor_tensor_scan(cum[:], dtf[:, gsl], zeros32[:],
                                                 0.0, op0=AL.add, op1=AL.add)
                    dst8 = pcm.tile([H, Q], FP32, tag="dst8")
                    nc.scalar.activation(dst8[:], cum[:], AF.Exp, scale=-1.0,
                                         bias=cum[:, Q - 1:Q])
                    sbt = pcm.tile([H, Q], FP32, tag="sbt")
                    nc.gpsimd.tensor_mul(sbt[:], dst8[:], dt16[:, gsl])

                    stk = pcm.tile([2 * H, Q], FP32, tag="stk")
                    nc.sync.dma_start(stk[0:H, :], cum[:])
                    nc.sync.dma_start(stk[H:2 * H, :], sbt[:])
                    ptr = ps_tr.tile([Q, 2 * H], FP32, tag="tr")
                    nc.tensor.transpose(ptr[:], stk[:], identf[0:2 * H, 0:2 * H])
                    ctall = pcm.tile([Q, 2 * H], FP32, tag="ctall")
                    nc.scalar.copy(ctall[:], ptr[:])
                    negcum = pcm.tile([Q, H], FP32, tag="negcum")
                    nc.vector.tensor_scalar(negcum[:], ctall[:, 0:H], -1.0,
                                            None, op0=AL.mult)

                    ptb = ps_tr.tile([Q, NST], BF16, tag="tr")
                    nc.tensor.transpose(ptb[:], cv[0:NST, 16, qsl],
                                        identb[0:NST, 0:NST])
                    btok = pcm.tile([Q, NST], BF16, tag="btok")
                    nc.scalar.copy(btok[:], ptb[:])

                    g0 = ps_tr.tile([Q, Q], FP32, tag="tr")
                    nc.tensor.matmul(g0[:], cv[0:NST, 16, qsl], ctc[:, qsl])
                    g0sb = pcm.tile([Q, Q], BF16, tag="g0sb")
                    nc.scalar.copy(g0sb[:], g0[:])

                    xtok = pxt.tile([Q, E], BF16, tag="xtok")
                    for f4 in range(ET // 4):
                        ptx = ps_tr.tile([Q, 4 * 128], BF16, tag="tr")
                        for j in range(4):
                            nc.tensor.transpose(
                                ptx[:, j * 128:(j + 1) * 128],
                                cv[:, 4 * f4 + j, qsl], identb[:])
                        nc.vector.tensor_copy(
                            xtok[:, f4 * 512:(f4 + 1) * 512], ptx[:])

                    for hg in range(H // 8):
                        h0 = hg * 8
                        stg = pstg.tile([1, 8 * Q], FP32, tag="stg")
                        nc.sync.dma_start(stg[:], cum[h0:h0 + 8, :])
                        bcq8 = pb2.tile([Q, 8 * Q], FP32, tag="bcq")
                        nc.gpsimd.partition_broadcast(bcq8[:], stg[:])
                        bce8 = pb1.tile([Q, 8 * Q], FP32, tag="bce")
                        nc.scalar.activation(bce8[:], bcq8[:], AF.Exp)
                        # mask AFTER bce8 snapshot (WAR dep keeps order)
                        nc.gpsimd.tensor_add(bcq8[:], bcq8[:], mask8[:])
                        for kp in range(4):
                            ftp = (h0 + 2 * kp) // 2
                            yp2 = ps_yp.tile([128, Q], FP32, tag="yp")
                            for k2 in range(2):
                                k = 2 * kp + k2
                                h = h0 + k
                                ro = k2 * 64
                                csl = slice(h * HD, (h + 1) * HD)
                                ksl = slice(k * Q, (k + 1) * Q)
                                lt = psc.tile([Q, Q], BF16, tag="lt")
                                nc.scalar.activation(lt[:], bcq8[:, ksl],
                                                     AF.Exp,
                                                     bias=negcum[:, h:h + 1])
                                mt_t = psc.tile([Q, Q], BF16, tag="mt")
                                nc.gpsimd.tensor_mul(mt_t[:], g0sb[:], lt[:])
                                xdt = psc.tile([Q, HD], BF16, tag="xdt")
                                nc.vector.tensor_scalar(
                                    xdt[:], xtok[:, csl],
                                    dttok[:, cidx * H + h:cidx * H + h + 1],
                                    None, op0=AL.mult)
                                cpos = psc.tile([NST, Q], BF16, tag="cpos")
                                nc.gpsimd.tensor_mul(cpos[:], ctc[:, qsl],
                                                     bce8[0:NST, ksl])
                                bh = psc.tile([Q, NST], BF16, tag="bh")
                                nc.vector.tensor_scalar(
                                    bh[:], btok[:],
                                    ctall[:, H + h:H + h + 1],
                                    None, op0=AL.mult)
                                nc.tensor.matmul(yp2[ro:ro + 64, :],
                                                 xdt[:], mt_t[:],
                                                 start=True, stop=False)
                                nc.tensor.matmul(yp2[ro:ro + 64, :],
                                                 S[:, csl], cpos[:],
                                                 start=False, stop=True)
                                sp = ps_sp.tile([NST, HD], FP32, tag="sp")
                                nc.tensor.matmul(sp[:], bh[:], xtok[:, csl])
                                nc.vector.scalar_tensor_tensor(
                                    S[:, csl], S[:, csl],
                                    bce8[0:NST, (k + 1) * Q - 1:(k + 1) * Q],
                                    sp[:], op0=AL.mult, op1=AL.add)
                            nc.vector.scalar_tensor_tensor(
                                g_sb[:, ftp, qsl], cv[:, ftp, qsl],
                                dxt[:, ftp:ftp + 1], yp2[:],
                                op0=AL.mult, op1=AL.add)

                # ---- gating, sum of squares, out_proj (scale deferred)
                nc.gpsimd.tensor_tensor(g_sb[:], g_sb[:], sz[:], op=AL.mult)
                ssq = ps_mm.tile([1, BLK], FP32, tag="mm")
                for ft in range(ET):
                    g2 = psm.tile([128, BLK], BF16, tag="th")
                    nc.vector.tensor_tensor(g2[:], g_sb[:, ft, :],
                                            g_sb[:, ft, :], op=AL.mult)
                    nc.tensor.matmul(ssq[:], onesb[:], g2[:],
                                     start=(ft == 0), stop=(ft == ET - 1))
                nc.vector.tensor_scalar(mall[0:1, tsl], ssq[:], 1.0 / E, EPS,
                                        op0=AL.mult, op1=AL.add)
                for mt in range(ET // 2):
                    ho = ps_mm.tile([128, BLK], FP32, tag="mm")
                    for kt in range(ET):
                        nc.tensor.matmul(ho[:],
                                         w_out[:, kt, mt * 128:(mt + 1) * 128],
                                         g_sb[:, kt, :],
                                         start=(kt == 0), stop=(kt == ET - 1))
                    hob = psm.tile([128, BLK], BF16, tag="hob")
                    nc.vector.tensor_copy(hob[:], ho[:])
                    nc.sync.dma_start(ho_d[:, mt, tsl], hob[:])

            # ---- layer epilogue: rmsnorm scale + selu (+ spill / pooling)
            nc.scalar.activation(mall[:], mall[:], AF.Ln)
            nc.scalar.activation(mall[:], mall[:], AF.Exp, scale=-0.5)
            for b in range(NBLK):
                tsl = slice(b * BLK, (b + 1) * BLK)
                scb = psm.tile([128, BLK], BF16, tag="scb")
                nc.gpsimd.partition_broadcast(scb[:], mall[0:1, tsl])
                for mt in range(ET // 2):
                    hot = pio.tile([128, BLK], BF16, tag="hot")
                    nc.sync.dma_start(hot[:], ho_d[:, mt, tsl])
                    nc.vector.tensor_mul(hot[:], hot[:], scb[:])
                    rl = psm.tile([128, BLK], BF16, tag="rl")
                    nc.scalar.activation(rl[:], hot[:], AF.Relu, scale=SELU_L)
                    ex = psm.tile([128, BLK], BF16, tag="ex")
                    nc.scalar.activation(ex[:], hot[:], AF.Exp, bias=lnla_t[:])
                    nc.vector.tensor_scalar(ex[:], ex[:], SELU_LA, SELU_LA,
                                            op0=AL.min, op1=AL.subtract)
                    nc.vector.tensor_tensor(rl[:], rl[:], ex[:], op=AL.add)
                    if layer == 0:
                        nc.sync.dma_start(u2[:, mt, tsl], rl[:])
                    else:
                        red = psm.tile([128, 1], FP32, tag="red")
                        nc.vector.tensor_reduce(red[:], rl[:],
                                                axis=mybir.AxisListType.X,
                                                op=AL.add)
                        nc.vector.tensor_add(pacc[:, mt:mt + 1],
                                             pacc[:, mt:mt + 1], red[:])

        pooled = psm.tile([128, KT_D], FP32, tag="pooled")
        nc.vector.tensor_scalar(pooled[:], pacc[:, 0:KT_D], 1.0 / L, None,
                                op0=AL.mult)
        ph = ps_sp.tile([1, 10], FP32, tag="sp")
        for kt in range(KT_D):
            nc.tensor.matmul(ph[:], pooled[:, kt:kt + 1], whead[:, kt, :],
                             start=(kt == 0), stop=(kt == KT_D - 1))
        ot = psm.tile([1, 10], FP32, tag="ot")
        nc.vector.tensor_add(ot[:], ph[:], bcat[:])
        nc.sync.dma_start(out_d[:], ot[:])

    nc.compile()
    return nc


def _host_inputs(inputs):
    m = {}
    m.update(_prep_layer(inputs, "1"))
    m.update(_prep_layer(inputs, "2"))
    j = np.arange(128)
    mneg = np.where(j[None, :] >= j[:, None], 0.0, -1e30)
    m["mask8"] = _bf(np.tile(mneg, (1, 8)))
    m["identb"] = _bf(np.eye(128))
    m["identf"] = _f32(np.eye(128))
    m["onesb"] = _bf(np.ones((128, 1)))
    wcat = np.concatenate([np.asarray(inputs["w_emo"], np.float32),
                           np.asarray(inputs["w_sen"], np.float32)], 0)
    m["whead"] = _f32(wcat.T.reshape(KT_D, 128, 10).transpose(1, 0, 2))
    m["bcat"] = _f32(np.concatenate([inputs["b_emo"], inputs["b_sen"]])
                     .reshape(1, 10))
    return m


def kernel(**inputs) -> np.ndarray:
    if "nc" not in _CACHE:
        _CACHE["nc"] = _build()
    nc = _CACHE["nc"]

    x = np.asarray(inputs["x"], np.float32)
    shared = _host_inputs(inputs)
    in_maps = []
    for s in range(NCORE):
        m = dict(shared)
        xts = x[s].T.reshape(KT_D, 128, L).transpose(1, 0, 2)
        m["xt"] = _bf(xts)
        in_maps.append(m)

    res = run_bass_kernel_spmd(nc, in_maps, core_ids=list(range(NCORE)))
    out = np.concatenate([r["out"] for r in res.results], 0)
    return out.astype(np.float32)


# revision 49
# speedup vs baseline: 1.4831x; 1.1222x over previous
"""Trainium2 Bass kernel for nn_AudioModelM1 (2x Mamba2 + selu + pool + heads).

Sharding: data-parallel over batch — 8 samples -> 8 NeuronCores, one sample per
core, no collectives.  Per-core layout is feature-major (features on SBUF
partitions, tokens on the free dim).  The selective scan uses the chunked
(quadratic-intra / recurrent-inter) Mamba2 formulation with Q=128 token chunks
so all heavy math runs on the TensorEngine.

Engine-balance notes (CoreSim cost model):
  - dt (softplus) is hoisted to a per-layer preamble and the RMSNorm scale +
    SELU are deferred to a per-layer epilogue so Ln never interleaves with the
    Exp/Tanh activations inside the block loop (act-table reloads are 1.3us).
  - silu(x) = x * (0.5*tanh(x/2) + 0.5): Tanh lives in the same activation
    table as Exp, so no table switches and no DVE reciprocal.
  - Small elementwise work in the scan inner loop runs on the Pool engine
    (flat 0.833 ns/elem, no access penalty); DVE ops keep all tensor operands
    packed bf16 in SBUF to hit the 2x/4x DVE perf modes.
  - Decay matrices are built per 4-head group: Pool stt folds (-cum_s + mask)
    so the Act exp runs batched over [Q, 4Q].
"""
import sys
sys.path.insert(0, "/opt/trn_rl_repo")

from contextlib import ExitStack

import numpy as np
import ml_dtypes

import concourse.bass as bass
import concourse.tile as tile
from concourse import bacc, mybir
from concourse.bass_utils import run_bass_kernel_spmd

FP32 = mybir.dt.float32
BF16 = mybir.dt.bfloat16
AL = mybir.AluOpType
AF = mybir.ActivationFunctionType

D = 1024
E = 2048
NST = 64
HD = 64
H = 32
DCONV = 4
CCH = E + 2 * NST             # 2176 conv channels (17 tiles)
F = 2 * E + 2 * NST + H       # 4256 in_proj rows
L = 2048
NCORE = 8

BLK = 256
NBLK = L // BLK
Q = 128
QPB = BLK // Q

KT_D = D // 128
MT_F = 34
CT = CCH // 128
ET = E // 128
HP = 4

SELU_L = 1.0507009873554805
SELU_A = 1.6732632423543772
SELU_LA = SELU_L * SELU_A
LN_LA = float(np.log(SELU_LA))
EPS = 1e-5

_CACHE = {}


def _bf(x):
    return np.ascontiguousarray(np.asarray(x, np.float32).astype(ml_dtypes.bfloat16))


def _f32(x):
    return np.ascontiguousarray(np.asarray(x, np.float32))


def _prep_layer(w, suf):
    in_w = np.asarray(w["in_proj_w" + suf], np.float32)
    out_w = np.asarray(w["out_proj_w" + suf], np.float32)
    norm_w = np.asarray(w["norm_w" + suf], np.float32)
    conv_w = np.asarray(w["conv_w" + suf], np.float32)
    conv_b = np.asarray(w["conv_b" + suf], np.float32)
    dt_b = np.asarray(w["dt_bias" + suf], np.float32)
    A_log = np.asarray(w["A_log" + suf], np.float32)
    Dp = np.asarray(w["D" + suf], np.float32)

    win = in_w.T.reshape(KT_D, 128, F).transpose(1, 0, 2)
    wo = (out_w * norm_w[None, :]).T
    wout = wo.reshape(ET, 128, D).transpose(1, 0, 2)
    cw = conv_w.reshape(CT, 128, DCONV).transpose(1, 0, 2)
    cb = conv_b.reshape(CT, 128).T
    dx = np.repeat(Dp, HD).reshape(ET, 128).T
    return {
        "win" + suf: _bf(win),
        "wout" + suf: _bf(wout),
        "cw" + suf: _f32(cw),
        "cb" + suf: _f32(cb),
        "dtb" + suf: _f32(dt_b.reshape(H, 1)),
        "A" + suf: _f32(-np.exp(A_log).reshape(H, 1)),
        "dx" + suf: _f32(dx),
    }


def _build():
    nc = bacc.Bacc("TRN2")
    dram = {}

    def din(name, shape, dt):
        dram[name] = nc.dram_tensor(name, list(shape), dt, kind="ExternalInput")
        return dram[name]

    xt = din("xt", (128, KT_D, L), BF16)
    for suf in ("1", "2"):
        din("win" + suf, (128, KT_D, F), BF16)
        din("wout" + suf, (128, ET, D), BF16)
        din("cw" + suf, (128, CT, DCONV), FP32)
        din("cb" + suf, (128, CT), FP32)
        din("dtb" + suf, (H, 1), FP32)
        din("A" + suf, (H, 1), FP32)
        din("dx" + suf, (128, ET), FP32)
    din("mask8", (128, 8 * Q), BF16)
    din("identb", (128, 128), BF16)
    din("identf", (128, 128), FP32)
    din("onesb", (128, 1), BF16)
    din("whead", (128, KT_D, 10), FP32)
    din("bcat", (1, 10), FP32)

    u2 = nc.dram_tensor("u2spill", [128, KT_D, L], BF16)
    ho_d = nc.dram_tensor("hospill", [128, KT_D, L], BF16)
    out_d = nc.dram_tensor("out", [1, 10], FP32, kind="ExternalOutput")

    with nc.allow_low_precision(reason="bf16 staging is intentional"), \
            tile.TileContext(nc) as tc, ExitStack() as ctx:
        pw = ctx.enter_context(tc.tile_pool(name="weights", bufs=1))
        pconst = ctx.enter_context(tc.tile_pool(name="consts", bufs=1))
        pio = ctx.enter_context(tc.tile_pool(name="io", bufs=2))
        pz = ctx.enter_context(tc.tile_pool(name="zsil", bufs=1))
        pxbc = ctx.enter_context(tc.tile_pool(name="xbcin", bufs=1))
        pxc = ctx.enter_context(tc.tile_pool(name="xconv", bufs=1))
        pg = ctx.enter_context(tc.tile_pool(name="gate", bufs=1))
        psc = ctx.enter_context(tc.tile_pool(name="scan", bufs=2))
        pxt = ctx.enter_context(tc.tile_pool(name="xtok", bufs=1))
        pcm = ctx.enter_context(tc.tile_pool(name="chunkmeta", bufs=1))
        pb1 = ctx.enter_context(tc.tile_pool(name="bcq1", bufs=2))
        pb2 = ctx.enter_context(tc.tile_pool(name="bcq2", bufs=2))
        psm = ctx.enter_context(tc.tile_pool(name="small", bufs=2))
        pstate = ctx.enter_context(tc.tile_pool(name="state", bufs=1))
        pdt = ctx.enter_context(tc.tile_pool(name="dtpre", bufs=1))
        pcv1 = ctx.enter_context(tc.tile_pool(name="cv1", bufs=1))

        ps_mm = ctx.enter_context(tc.tile_pool(name="psmm", bufs=2, space="PSUM"))
        ps_tr = ctx.enter_context(tc.tile_pool(name="pstr", bufs=2, space="PSUM"))
        ps_yp = ctx.enter_context(tc.tile_pool(name="psyp", bufs=2, space="PSUM"))
        ps_sp = ctx.enter_context(tc.tile_pool(name="pssp", bufs=2, space="PSUM"))

        w_in = pw.tile([128, KT_D, F], BF16)
        w_out = pw.tile([128, ET, D], BF16)
        cw = pw.tile([128, CT, DCONV], FP32)
        cb = pw.tile([128, CT], FP32)
        dtb = pw.tile([H, 1], FP32)
        Atile = pw.tile([H, 1], FP32)
        dxt = pw.tile([128, ET], FP32)

        mask8 = pconst.tile([128, 8 * Q], BF16)
        identb = pconst.tile([128, 128], BF16)
        identf = pconst.tile([3 * H, 3 * H], FP32)
        onesb = pconst.tile([128, 1], BF16)
        whead = pconst.tile([128, KT_D, 10], FP32)
        bcat = pconst.tile([1, 10], FP32)
        zeros32 = pconst.tile([H, Q], FP32)
        ones32 = pconst.tile([H, 1], FP32)
        lnla_t = pconst.tile([128, 1], FP32)

        S = pstate.tile([NST, H * HD], BF16)   # [n, (h,p)] heads at base part 0
        pacc = pstate.tile([128, ET], FP32)

        # per-layer hoisted dt: raw -> exp -> (ln in place) -> dt; then alog
        dtf = pdt.tile([H, L], FP32)     # exp(raw+b) -> dt (f32) -> alog=dt*A
        lndttok = pdt.tile([Q, NBLK * QPB * H], FP32)  # token-major ln(dt)
        mall = pdt.tile([1, L], BF16)    # ssq/E+eps -> ln -> rmsnorm scale
        tailt = pdt.tile([128, CT, HP - 1], BF16)  # conv tail carry

        for t, name in ((mask8, "mask8"), (identb, "identb"),
                        (onesb, "onesb"), (whead, "whead"), (bcat, "bcat")):
            nc.sync.dma_start(t[:], dram[name][:])
        nc.sync.dma_start(identf[:], dram["identf"][0:3 * H, 0:3 * H])
        nc.vector.memset(zeros32[:], 0.0)
        nc.vector.memset(pacc[:], 0.0)
        nc.vector.memset(ones32[:], 1.0)
        nc.vector.memset(lnla_t[:], LN_LA)

        for layer in (0, 1):
            suf = "12"[layer]
            for t, name in ((w_in, "win"), (w_out, "wout"), (cw, "cw"), (cb, "cb"),
                            (dtb, "dtb"), (Atile, "A"), (dxt, "dx")):
                nc.sync.dma_start(t[:], dram[name + suf][:])
            nc.vector.memset(S[:], 0.0)

            src = xt if layer == 0 else u2

            # ---- dt preamble: dt_raw for the whole layer, one softplus ----
            for b in range(NBLK):
                tsl = slice(b * BLK, (b + 1) * BLK)
                u_t = pio.tile([128, KT_D, BLK], BF16, tag="ut")
                nc.sync.dma_start(u_t[:], src[:, :, tsl])
                pmmd = ps_mm.tile([H, BLK], FP32, tag="mm")
                for kt in range(KT_D):
                    nc.tensor.matmul(pmmd[:], w_in[:, kt, F - H:F],
                                     u_t[:, kt, :], start=(kt == 0),
                                     stop=(kt == KT_D - 1))
                nc.scalar.activation(dtf[:, tsl], pmmd[:], AF.Exp, bias=dtb[:])
            # softplus ln over the whole layer at once
            nc.scalar.activation(dtf[:], dtf[:], AF.Ln, bias=ones32[:])
            # token-major ln(dt) columns (still inside the Ln table window):
            # per-head decay matrices get dt folded in via the exp bias
            for c in range(NBLK * QPB):
                ptd = ps_tr.tile([Q, H], FP32, tag="tr")
                nc.tensor.transpose(ptd[:], dtf[:, c * Q:(c + 1) * Q],
                                    identf[0:H, 0:H])
                nc.scalar.activation(lndttok[:, c * H:(c + 1) * H], ptd[:],
                                     AF.Ln)
            nc.vector.tensor_scalar(dtf[:], dtf[:], Atile[:], None, op0=AL.mult)
            # dtf now holds alog = dt * A

            xbc = pxbc.tile([128, CT, BLK + HP], BF16)
            nc.vector.memset(xbc[:, :, 0:HP], 0.0)

            ut_store = {}

            def prep_inproj_xbc(b):
                """DMA u_t(b) now; return emit-closures for the xbc half of
                in_proj(b) to be woven into the previous block's scan."""
                u_t = pio.tile([128, KT_D, BLK], BF16, tag="ut")
                ut_store[b] = u_t
                nc.sync.dma_start(u_t[:], src[:, :, b * BLK:(b + 1) * BLK])
                steps = []
                if b > 0:
                    steps.append(lambda: nc.vector.tensor_copy(
                        tailt[:], xbc[:, :, BLK + 1:BLK + HP]))

                def mk(mt):
                    def go():
                        pmm = ps_mm.tile([128, BLK], FP32, tag="mm")
                        for kt in range(KT_D):
                            nc.tensor.matmul(
                                pmm[:], w_in[:, kt, mt * 128:(mt + 1) * 128],
                                u_t[:, kt, :], start=(kt == 0),
                                stop=(kt == KT_D - 1))
                        nc.vector.tensor_copy(xbc[:, mt - ET, HP:HP + BLK],
                                              pmm[:])
                    return go
                for mt in range(ET, 33):
                    steps.append(mk(mt))
                if b > 0:
                    steps.append(lambda: nc.vector.tensor_copy(
                        xbc[:, :, 1:HP], tailt[:]))
                return steps

            def emit_z(b, ho_iter=None):
                """z half of in_proj(b) -> sz; optionally interleave the
                previous block's out_proj groups from ho_iter."""
                u_t = ut_store.pop(b)
                sz = pz.tile([128, ET, BLK], BF16)
                sz_store[b] = sz
                for mt in range(ET):
                    if ho_iter is not None:
                        nxt = next(ho_iter, None)
                        if nxt is not None:
                            nxt()
                    pmm = ps_mm.tile([128, BLK], FP32, tag="mm")
                    for kt in range(KT_D):
                        nc.tensor.matmul(
                            pmm[:], w_in[:, kt, mt * 128:(mt + 1) * 128],
                            u_t[:, kt, :], start=(kt == 0), stop=(kt == KT_D - 1))
                    # silu(z) = z * (0.5*tanh(z/2) + 0.5)
                    th = psm.tile([128, BLK], BF16, tag="th")
                    nc.scalar.activation(th[:], pmm[:], AF.Tanh, scale=0.5)
                    zb = psm.tile([128, BLK], BF16, tag="zb")
                    nc.scalar.copy(zb[:], pmm[:])
                    nc.vector.tensor_scalar(th[:], th[:], 0.5, 0.5,
                                            op0=AL.mult, op1=AL.add)
                    nc.gpsimd.tensor_mul(sz[:, mt, :], th[:], zb[:])

            def emit_conv(b):
                # causal depthwise conv (+bias): 4 DVE tensor-scalar products
                # (4x perf mode) + 3 Pool tensor-tensor adds, then tanh-silu
                cv = pxc.tile([128, CT, BLK], BF16)
                cv_store[b] = cv
                for ct in range(CT):
                    cvs = cv[:, ct, :]
                    cvt = pcv1.tile([128, 3, BLK], BF16, tag="cvt")
                    nc.vector.tensor_scalar(cvs, xbc[:, ct, 1:1 + BLK],
                                            cw[:, ct, 0:1], cb[:, ct:ct + 1],
                                            op0=AL.mult, op1=AL.add)
                    for k in range(1, DCONV):
                        nc.vector.tensor_scalar(cvt[:, k - 1, :],
                                                xbc[:, ct, 1 + k:1 + k + BLK],
                                                cw[:, ct, k:k + 1], None,
                                                op0=AL.mult)
                    nc.gpsimd.tensor_add(cvt[:, 0, :], cvt[:, 0, :],
                                         cvt[:, 1, :])
                    nc.gpsimd.tensor_add(cvs, cvs, cvt[:, 2, :])
                    nc.gpsimd.tensor_add(cvs, cvs, cvt[:, 0, :])
                for cp in ((0, 4), (4, 4), (8, 4), (12, 4), (16, 1)):
                    c0, n_in = cp
                    w = n_in * BLK
                    cvp = cv[:, c0:c0 + n_in, :]
                    th2 = pcm.tile([128, 4 * BLK], BF16, tag="th2")
                    nc.scalar.activation(th2[:, 0:w], cvp, AF.Tanh, scale=0.5)
                    nc.vector.tensor_scalar(th2[:, 0:w], th2[:, 0:w], 0.5, 0.5,
                                            op0=AL.mult, op1=AL.add)
                    nc.gpsimd.tensor_tensor(cvp, th2[:, 0:w], cvp, op=AL.mult)
                # cv[:, 0:16] = x (silu'd), cv[:, 16] = B (parts 0:64) | C
                ctc = psm.tile([NST, BLK], BF16, tag="ctc")
                ctc_store[b] = ctc
                nc.sync.dma_start(ctc[:], cv[NST:128, 16, :])

            sz_store = {}
            cv_store = {}
            ctc_store = {}

            prologue = prep_inproj_xbc(0)
            for fn in prologue:
                fn()
            emit_conv(0)
            emit_z(0)

            for b in range(NBLK):
                tsl = slice(b * BLK, (b + 1) * BLK)
                cv = cv_store.pop(b)
                ctc = ctc_store.pop(b)
                steal = iter(prep_inproj_xbc(b + 1) if b + 1 < NBLK else [])

                g_sb = pg.tile([128, ET, BLK], BF16)

                for qi in range(QPB):
                    qsl = slice(qi * Q, (qi + 1) * Q)
                    gsl = slice(b * BLK + qi * Q, b * BLK + (qi + 1) * Q)
                    cidx = b * QPB + qi
                    cum = pcm.tile([H, Q], FP32, tag="cum")
                    nc.vector.tensor_tensor_scan(cum[:], dtf[:, gsl], zeros32[:],
                                                 0.0, op0=AL.add, op1=AL.add)
                    ptr = ps_tr.tile([Q, H], FP32, tag="tr")
                    nc.tensor.transpose(ptr[:], cum[:], identf[0:H, 0:H])
                    ctall = pcm.tile([Q, H], FP32, tag="ctall")
                    nc.scalar.copy(ctall[:], ptr[:])
                    # exp bias per head: ln(dt_s) - cum_s  (dt folded into lt)
                    negcl = pcm.tile([Q, H], FP32, tag="negcl")
                    nc.vector.tensor_sub(negcl[:],
                                         lndttok[:, cidx * H:(cidx + 1) * H],
                                         ctall[:])

                    ptb = ps_tr.tile([Q, NST], BF16, tag="tr")
                    nc.tensor.transpose(ptb[:], cv[0:NST, 16, qsl],
                                        identb[0:NST, 0:NST])
                    btok = pcm.tile([Q, NST], BF16, tag="btok")
                    nc.scalar.copy(btok[:], ptb[:])

                    g0 = ps_tr.tile([Q, Q], FP32, tag="tr")
                    nc.tensor.matmul(g0[:], cv[0:NST, 16, qsl], ctc[:, qsl])
                    g0sb = pcm.tile([Q, Q], BF16, tag="g0sb")
                    nc.scalar.copy(g0sb[:], g0[:])

                    xtok = pxt.tile([Q, E], BF16, tag="xtok")
                    for f4 in range(ET // 4):
                        ptx = ps_tr.tile([Q, 4 * 128], BF16, tag="tr")
                        for j in range(4):
                            nc.tensor.transpose(
                                ptx[:, j * 128:(j + 1) * 128],
                                cv[:, 4 * f4 + j, qsl], identb[:])
                        nc.vector.tensor_copy(
                            xtok[:, f4 * 512:(f4 + 1) * 512], ptx[:])

                    for hg in range(H // 8):
                        h0 = hg * 8
                        bcq8 = pb2.tile([Q, 8 * Q], FP32, tag="bcq")
                        for hh in range(2):
                            stg = psc.tile([1, 4 * Q], FP32, tag="stg")
                            nc.sync.dma_start(
                                stg[:], cum[h0 + 4 * hh:h0 + 4 * hh + 4, :])
                            nc.gpsimd.partition_broadcast(
                                bcq8[:, 4 * hh * Q:4 * (hh + 1) * Q], stg[:])
                        bce8 = pb1.tile([Q, 8 * Q], FP32, tag="bce")
                        nc.scalar.activation(bce8[:], bcq8[:], AF.Exp)
                        # mask AFTER bce8 snapshot (WAR dep keeps order)
                        nc.gpsimd.tensor_add(bcq8[:], bcq8[:], mask8[:])
                        for kp in range(4):
                            ftp = (h0 + 2 * kp) // 2
                            yp2 = ps_yp.tile([128, Q], FP32, tag="yp")
                            for k2 in range(2):
                                k = 2 * kp + k2
                                h = h0 + k
                                ro = k2 * 64
                                csl = slice(h * HD, (h + 1) * HD)
                                ksl = slice(k * Q, (k + 1) * Q)
                                lt = psc.tile([Q, Q], FP32, tag="lt")
                                nc.scalar.activation(lt[:], bcq8[:, ksl],
                                                     AF.Exp,
                                                     bias=negcl[:, h:h + 1])
                                mt_t = psc.tile([Q, Q], BF16, tag="mt")
                                nc.gpsimd.tensor_mul(mt_t[:], g0sb[:], lt[:])
                                cpos = psc.tile([NST, Q], BF16, tag="cpos")
                                nc.gpsimd.tensor_mul(cpos[:], ctc[:, qsl],
                                                     bce8[0:NST, ksl])
                                bh = psc.tile([Q, NST], BF16, tag="bh")
                                nc.vector.tensor_scalar(
                                    bh[:], btok[:],
                                    lt[:, Q - 1:Q],
                                    None, op0=AL.mult)
                                nc.tensor.matmul(yp2[ro:ro + 64, :],
                                                 xtok[:, csl], mt_t[:],
                                                 start=True, stop=False)
                                nc.tensor.matmul(yp2[ro:ro + 64, :],
                                                 S[:, csl], cpos[:],
                                                 start=False, stop=True)
                                sp = ps_sp.tile([NST, HD], FP32, tag="sp")
                                nc.tensor.matmul(sp[:], bh[:], xtok[:, csl])
                                nc.vector.scalar_tensor_tensor(
                                    S[:, csl], S[:, csl],
                                    bce8[0:NST, (k + 1) * Q - 1:(k + 1) * Q],
                                    sp[:], op0=AL.mult, op1=AL.add)
                            nc.vector.scalar_tensor_tensor(
                                g_sb[:, ftp, qsl], cv[:, ftp, qsl],
                                dxt[:, ftp:ftp + 1], yp2[:],
                                op0=AL.mult, op1=AL.add)

                # ---- gating, sum of squares, out_proj (scale deferred)
                nc.gpsimd.tensor_tensor(g_sb[:], g_sb[:], sz[:], op=AL.mult)
                ssq = ps_mm.tile([1, BLK], FP32, tag="mm")
                for ft in range(ET):
                    g2 = psm.tile([128, BLK], BF16, tag="th")
                    nc.vector.tensor_tensor(g2[:], g_sb[:, ft, :],
                                            g_sb[:, ft, :], op=AL.mult)
                    nc.tensor.matmul(ssq[:], onesb[:], g2[:],
                                     start=(ft == 0), stop=(ft == ET - 1))
                nc.vector.tensor_scalar(mall[0:1, tsl], ssq[:], 1.0 / E, EPS,
                                        op0=AL.mult, op1=AL.add)
                for mt in range(ET // 2):
                    ho = ps_mm.tile([128, BLK], FP32, tag="mm")
                    for kt in range(ET):
                        nc.tensor.matmul(ho[:],
                                         w_out[:, kt, mt * 128:(mt + 1) * 128],
                                         g_sb[:, kt, :],
                                         start=(kt == 0), stop=(kt == ET - 1))
                    hob = psm.tile([128, BLK], BF16, tag="hob")
                    nc.vector.tensor_copy(hob[:], ho[:])
                    nc.sync.dma_start(ho_d[:, mt, tsl], hob[:])

            # ---- layer epilogue: rmsnorm scale + selu (+ spill / pooling)
            nc.scalar.activation(mall[:], mall[:], AF.Ln)
            nc.scalar.activation(mall[:], mall[:], AF.Exp, scale=-0.5)
            for b in range(NBLK):
                tsl = slice(b * BLK, (b + 1) * BLK)
                scb = psm.tile([128, BLK], BF16, tag="scb")
                nc.gpsimd.partition_broadcast(scb[:], mall[0:1, tsl])
                for mt in range(ET // 2):
                    hot = pio.tile([128, BLK], BF16, tag="hot")
                    nc.sync.dma_start(hot[:], ho_d[:, mt, tsl])
                    nc.vector.tensor_mul(hot[:], hot[:], scb[:])
                    rl = psm.tile([128, BLK], BF16, tag="rl")
                    nc.scalar.activation(rl[:], hot[:], AF.Relu, scale=SELU_L)
                    ex = psm.tile([128, BLK], BF16, tag="ex")
                    nc.scalar.activation(ex[:], hot[:], AF.Exp, bias=lnla_t[:])
                    nc.vector.tensor_scalar(ex[:], ex[:], SELU_LA, SELU_LA,
                                            op0=AL.min, op1=AL.subtract)
                    nc.vector.tensor_tensor(rl[:], rl[:], ex[:], op=AL.add)
                    if layer == 0:
                        nc.sync.dma_start(u2[:, mt, tsl], rl[:])
                    else:
                        red = psm.tile([128, 1], FP32, tag="red")
                        nc.vector.tensor_reduce(red[:], rl[:],
                                                axis=mybir.AxisListType.X,
                                                op=AL.add)
                        nc.vector.tensor_add(pacc[:, mt:mt + 1],
                                             pacc[:, mt:mt + 1], red[:])

        pooled = psm.tile([128, KT_D], FP32, tag="pooled")
        nc.vector.tensor_scalar(pooled[:], pacc[:, 0:KT_D], 1.0 / L, None,
                                op0=AL.mult)
        ph = ps_sp.tile([1, 10], FP32, tag="sp")
        for kt in range(KT_D):
            nc.tensor.matmul(ph[:], pooled[:, kt:kt + 1], whead[:, kt, :],
                             start=(kt == 0), stop=(kt == KT_D - 1))
        ot = psm.tile([1, 10], FP32, tag="ot")
        nc.vector.tensor_add(ot[:], ph[:], bcat[:])
        nc.sync.dma_start(out_d[:], ot[:])

    nc.compile()
    return nc


def _host_inputs(inputs):
    m = {}
    m.update(_prep_layer(inputs, "1"))
    m.update(_prep_layer(inputs, "2"))
    j = np.arange(128)
    mneg = np.where(j[None, :] >= j[:, None], 0.0, -1e30)
    m["mask8"] = _bf(np.tile(mneg, (1, 8)))
    m["identb"] = _bf(np.eye(128))
    m["identf"] = _f32(np.eye(128))
    m["onesb"] = _bf(np.ones((128, 1)))
    wcat = np.concatenate([np.asarray(inputs["w_emo"], np.float32),
                           np.asarray(inputs["w_sen"], np.float32)], 0)
    m["whead"] = _f32(wcat.T.reshape(KT_D, 128, 10).transpose(1, 0, 2))
    m["bcat"] = _f32(np.concatenate([inputs["b_emo"], inputs["b_sen"]])
                     .reshape(1, 10))
    return m


def kernel(**inputs) -> np.ndarray:
    if "nc" not in _CACHE:
        _CACHE["nc"] = _build()
    nc = _CACHE["nc"]

    x = np.asarray(inputs["x"], np.float32)
    shared = _host_inputs(inputs)
    in_maps = []
    for s in range(NCORE):
        m = dict(shared)
        xts = x[s].T.reshape(KT_D, 128, L).transpose(1, 0, 2)
        m["xt"] = _bf(xts)
        in_maps.append(m)

    res = run_bass_kernel_spmd(nc, in_maps, core_ids=list(range(NCORE)))
    out = np.concatenate([r["out"] for r in res.results], 0)
    return out.astype(np.float32)


# revision 51
# speedup vs baseline: 1.6747x; 1.1292x over previous
"""Trainium2 Bass kernel for nn_AudioModelM1 (2x Mamba2 + selu + pool + heads).

Sharding: data-parallel over batch — 8 samples -> 8 NeuronCores, one sample per
core, no collectives.  Per-core layout is feature-major (features on SBUF
partitions, tokens on the free dim).  The selective scan uses the chunked
(quadratic-intra / recurrent-inter) Mamba2 formulation with Q=128 token chunks
so all heavy math runs on the TensorEngine.

Engine-balance notes (CoreSim cost model):
  - dt (softplus) is hoisted to a per-layer preamble and the RMSNorm scale +
    SELU are deferred to a per-layer epilogue so Ln never interleaves with the
    Exp/Tanh activations inside the block loop (act-table reloads are 1.3us).
  - silu(x) = x * (0.5*tanh(x/2) + 0.5): Tanh lives in the same activation
    table as Exp, so no table switches and no DVE reciprocal.
  - Small elementwise work in the scan inner loop runs on the Pool engine
    (flat 0.833 ns/elem, no access penalty); DVE ops keep all tensor operands
    packed bf16 in SBUF to hit the 2x/4x DVE perf modes.
  - Decay matrices are built per 4-head group: Pool stt folds (-cum_s + mask)
    so the Act exp runs batched over [Q, 4Q].
"""
import sys
sys.path.insert(0, "/opt/trn_rl_repo")

from contextlib import ExitStack

import numpy as np
import ml_dtypes

import concourse.bass as bass
import concourse.tile as tile
from concourse import bacc, mybir
from concourse.bass_utils import run_bass_kernel_spmd

FP32 = mybir.dt.float32
BF16 = mybir.dt.bfloat16
AL = mybir.AluOpType
AF = mybir.ActivationFunctionType

D = 1024
E = 2048
NST = 64
HD = 64
H = 32
DCONV = 4
CCH = E + 2 * NST             # 2176 conv channels (17 tiles)
F = 2 * E + 2 * NST + H       # 4256 in_proj rows
L = 2048
NCORE = 8

BLK = 256
NBLK = L // BLK
Q = 128
QPB = BLK // Q

KT_D = D // 128
MT_F = 34
CT = CCH // 128
ET = E // 128
HP = 4

SELU_L = 1.0507009873554805
SELU_A = 1.6732632423543772
SELU_LA = SELU_L * SELU_A
LN_LA = float(np.log(SELU_LA))
EPS = 1e-5

_CACHE = {}


def _bf(x):
    return np.ascontiguousarray(np.asarray(x, np.float32).astype(ml_dtypes.bfloat16))


def _f32(x):
    return np.ascontiguousarray(np.asarray(x, np.float32))


def _prep_layer(w, suf):
    in_w = np.asarray(w["in_proj_w" + suf], np.float32)
    out_w = np.asarray(w["out_proj_w" + suf], np.float32)
    norm_w = np.asarray(w["norm_w" + suf], np.float32)
    conv_w = np.asarray(w["conv_w" + suf], np.float32)
    conv_b = np.asarray(w["conv_b" + suf], np.float32)
    dt_b = np.asarray(w["dt_bias" + suf], np.float32)
    A_log = np.asarray(w["A_log" + suf], np.float32)
    Dp = np.asarray(w["D" + suf], np.float32)

    win = in_w.T.reshape(KT_D, 128, F).transpose(1, 0, 2)
    wo = (out_w * norm_w[None, :]).T
    wout = wo.reshape(ET, 128, D).transpose(1, 0, 2)
    cw = conv_w.reshape(CT, 128, DCONV).transpose(1, 0, 2)
    cb = conv_b.reshape(CT, 128).T
    dx = np.repeat(Dp, HD).reshape(ET, 128).T
    return {
        "win" + suf: _bf(win),
        "wout" + suf: _bf(wout),
        "cw" + suf: _f32(cw),
        "cb" + suf: _f32(cb),
        "dtb" + suf: _f32(dt_b.reshape(H, 1)),
        "A" + suf: _f32(-np.exp(A_log).reshape(H, 1)),
        "dx" + suf: _f32(dx),
    }


def _build():
    nc = bacc.Bacc("TRN2")
    dram = {}

    def din(name, shape, dt):
        dram[name] = nc.dram_tensor(name, list(shape), dt, kind="ExternalInput")
        return dram[name]

    xt = din("xt", (128, KT_D, L), BF16)
    for suf in ("1", "2"):
        din("win" + suf, (128, KT_D, F), BF16)
        din("wout" + suf, (128, ET, D), BF16)
        din("cw" + suf, (128, CT, DCONV), FP32)
        din("cb" + suf, (128, CT), FP32)
        din("dtb" + suf, (H, 1), FP32)
        din("A" + suf, (H, 1), FP32)
        din("dx" + suf, (128, ET), FP32)
    din("mask8", (128, 8 * Q), BF16)
    din("identb", (128, 128), BF16)
    din("identf", (128, 128), FP32)
    din("onesb", (128, 1), BF16)
    din("whead", (128, KT_D, 10), FP32)
    din("bcat", (1, 10), FP32)

    u2 = nc.dram_tensor("u2spill", [128, KT_D, L], BF16)
    ho_d = nc.dram_tensor("hospill", [128, KT_D, L], BF16)
    out_d = nc.dram_tensor("out", [1, 10], FP32, kind="ExternalOutput")

    with nc.allow_low_precision(reason="bf16 staging is intentional"), \
            tile.TileContext(nc) as tc, ExitStack() as ctx:
        pw = ctx.enter_context(tc.tile_pool(name="weights", bufs=1))
        pconst = ctx.enter_context(tc.tile_pool(name="consts", bufs=1))
        pio = ctx.enter_context(tc.tile_pool(name="io", bufs=2))
        pz = ctx.enter_context(tc.tile_pool(name="zsil", bufs=1))
        pxbc = ctx.enter_context(tc.tile_pool(name="xbcin", bufs=1))
        pxc = ctx.enter_context(tc.tile_pool(name="xconv", bufs=1))
        pg = ctx.enter_context(tc.tile_pool(name="gate", bufs=1))
        psc = ctx.enter_context(tc.tile_pool(name="scan", bufs=2))
        pxt = ctx.enter_context(tc.tile_pool(name="xtok", bufs=1))
        pcm = ctx.enter_context(tc.tile_pool(name="chunkmeta", bufs=1))
        pb1 = ctx.enter_context(tc.tile_pool(name="bcq1", bufs=2))
        pb2 = ctx.enter_context(tc.tile_pool(name="bcq2", bufs=2))
        psm = ctx.enter_context(tc.tile_pool(name="small", bufs=2))
        pstate = ctx.enter_context(tc.tile_pool(name="state", bufs=1))
        pdt = ctx.enter_context(tc.tile_pool(name="dtpre", bufs=1))
        pcv1 = ctx.enter_context(tc.tile_pool(name="cv1", bufs=1))

        ps_mm = ctx.enter_context(tc.tile_pool(name="psmm", bufs=2, space="PSUM"))
        ps_tr = ctx.enter_context(tc.tile_pool(name="pstr", bufs=2, space="PSUM"))
        ps_yp = ctx.enter_context(tc.tile_pool(name="psyp", bufs=2, space="PSUM"))
        ps_sp = ctx.enter_context(tc.tile_pool(name="pssp", bufs=2, space="PSUM"))

        w_in = pw.tile([128, KT_D, F], BF16)
        w_out = pw.tile([128, ET, D], BF16)
        cw = pw.tile([128, CT, DCONV], FP32)
        cb = pw.tile([128, CT], FP32)
        dtb = pw.tile([H, 1], FP32)
        Atile = pw.tile([H, 1], FP32)
        dxt = pw.tile([128, ET], FP32)

        mask8 = pconst.tile([128, 8 * Q], BF16)
        identb = pconst.tile([128, 128], BF16)
        identf = pconst.tile([3 * H, 3 * H], FP32)
        onesb = pconst.tile([128, 1], BF16)
        whead = pconst.tile([128, KT_D, 10], FP32)
        bcat = pconst.tile([1, 10], FP32)
        zeros32 = pconst.tile([H, Q], FP32)
        ones32 = pconst.tile([H, 1], FP32)
        lnla_t = pconst.tile([128, 1], FP32)

        S = pstate.tile([NST, H * HD], BF16)   # [n, (h,p)] heads at base part 0
        pacc = pstate.tile([128, ET], FP32)

        # per-layer hoisted dt: raw -> exp -> (ln in place) -> dt; then alog
        dtf = pdt.tile([H, L], FP32)     # exp(raw+b) -> dt (f32) -> alog=dt*A
        lndttok = pdt.tile([Q, NBLK * QPB * H], FP32)  # token-major ln(dt)
        mall = pdt.tile([1, L], BF16)    # ssq/E+eps -> ln -> rmsnorm scale
        tailt = pdt.tile([128, CT, HP - 1], BF16)  # conv tail carry

        for t, name in ((mask8, "mask8"), (identb, "identb"),
                        (onesb, "onesb"), (whead, "whead"), (bcat, "bcat")):
            nc.sync.dma_start(t[:], dram[name][:])
        nc.sync.dma_start(identf[:], dram["identf"][0:3 * H, 0:3 * H])
        nc.vector.memset(zeros32[:], 0.0)
        nc.vector.memset(pacc[:], 0.0)
        nc.vector.memset(ones32[:], 1.0)
        nc.vector.memset(lnla_t[:], LN_LA)

        for layer in (0, 1):
            suf = "12"[layer]
            for t, name in ((w_in, "win"), (w_out, "wout"), (cw, "cw"), (cb, "cb"),
                            (dtb, "dtb"), (Atile, "A"), (dxt, "dx")):
                nc.sync.dma_start(t[:], dram[name + suf][:])
            nc.vector.memset(S[:], 0.0)

            src = xt if layer == 0 else u2

            # ---- dt preamble: dt_raw for the whole layer, one softplus ----
            for b in range(NBLK):
                tsl = slice(b * BLK, (b + 1) * BLK)
                u_t = pio.tile([128, KT_D, BLK], BF16, tag="ut")
                nc.sync.dma_start(u_t[:], src[:, :, tsl])
                pmmd = ps_mm.tile([H, BLK], FP32, tag="mm")
                for kt in range(KT_D):
                    nc.tensor.matmul(pmmd[:], w_in[:, kt, F - H:F],
                                     u_t[:, kt, :], start=(kt == 0),
                                     stop=(kt == KT_D - 1))
                nc.scalar.activation(dtf[:, tsl], pmmd[:], AF.Exp, bias=dtb[:])
            # softplus ln over the whole layer at once
            nc.scalar.activation(dtf[:], dtf[:], AF.Ln, bias=ones32[:])
            # token-major ln(dt) columns (still inside the Ln table window):
            # per-head decay matrices get dt folded in via the exp bias
            for c in range(NBLK * QPB):
                ptd = ps_tr.tile([Q, H], FP32, tag="tr")
                nc.tensor.transpose(ptd[:], dtf[:, c * Q:(c + 1) * Q],
                                    identf[0:H, 0:H])
                nc.scalar.activation(lndttok[:, c * H:(c + 1) * H], ptd[:],
                                     AF.Ln)
            nc.vector.tensor_scalar(dtf[:], dtf[:], Atile[:], None, op0=AL.mult)
            # dtf now holds alog = dt * A

            xbc = pxbc.tile([128, CT, BLK + HP], BF16)
            nc.vector.memset(xbc[:, :, 0:HP], 0.0)

            ut_store = {}

            def prep_inproj_xbc(b):
                """DMA u_t(b) now; return emit-closures for the xbc half of
                in_proj(b) to be woven into the previous block's scan."""
                u_t = pio.tile([128, KT_D, BLK], BF16, tag="ut")
                ut_store[b] = u_t
                nc.sync.dma_start(u_t[:], src[:, :, b * BLK:(b + 1) * BLK])
                steps = []
                if b > 0:
                    steps.append(lambda: nc.vector.tensor_copy(
                        tailt[:], xbc[:, :, BLK + 1:BLK + HP]))

                def mk(mt):
                    def go():
                        pmm = ps_mm.tile([128, BLK], FP32, tag="mm")
                        for kt in range(KT_D):
                            nc.tensor.matmul(
                                pmm[:], w_in[:, kt, mt * 128:(mt + 1) * 128],
                                u_t[:, kt, :], start=(kt == 0),
                                stop=(kt == KT_D - 1))
                        nc.vector.tensor_copy(xbc[:, mt - ET, HP:HP + BLK],
                                              pmm[:])
                    return go
                for mt in range(ET, 33):
                    steps.append(mk(mt))
                if b > 0:
                    steps.append(lambda: nc.vector.tensor_copy(
                        xbc[:, :, 1:HP], tailt[:]))
                return steps

            def emit_z(b, ho_iter=None):
                """z half of in_proj(b) -> sz; optionally interleave the
                previous block's out_proj groups from ho_iter."""
                u_t = ut_store.pop(b)
                sz = pz.tile([128, ET, BLK], BF16)
                sz_store[b] = sz
                for mt in range(ET):
                    if ho_iter is not None:
                        nxt = next(ho_iter, None)
                        if nxt is not None:
                            nxt()
                    pmm = ps_mm.tile([128, BLK], FP32, tag="mm")
                    for kt in range(KT_D):
                        nc.tensor.matmul(
                            pmm[:], w_in[:, kt, mt * 128:(mt + 1) * 128],
                            u_t[:, kt, :], start=(kt == 0), stop=(kt == KT_D - 1))
                    # silu(z) = z * (0.5*tanh(z/2) + 0.5)
                    th = psm.tile([128, BLK], BF16, tag="th")
                    nc.scalar.activation(th[:], pmm[:], AF.Tanh, scale=0.5)
                    zb = psm.tile([128, BLK], BF16, tag="zb")
                    nc.scalar.copy(zb[:], pmm[:])
                    nc.vector.tensor_scalar(th[:], th[:], 0.5, 0.5,
                                            op0=AL.mult, op1=AL.add)
                    nc.gpsimd.tensor_mul(sz[:, mt, :], th[:], zb[:])

            def emit_conv(b):
                # causal depthwise conv (+bias): 4 DVE tensor-scalar products
                # (4x perf mode) + 3 Pool tensor-tensor adds, then tanh-silu
                cv = pxc.tile([128, CT, BLK], BF16)
                cv_store[b] = cv
                for ct in range(CT):
                    cvs = cv[:, ct, :]
                    cvt = pcv1.tile([128, 3, BLK], BF16, tag="cvt")
                    nc.vector.tensor_scalar(cvs, xbc[:, ct, 1:1 + BLK],
                                            cw[:, ct, 0:1], cb[:, ct:ct + 1],
                                            op0=AL.mult, op1=AL.add)
                    for k in range(1, DCONV):
                        nc.vector.tensor_scalar(cvt[:, k - 1, :],
                                                xbc[:, ct, 1 + k:1 + k + BLK],
                                                cw[:, ct, k:k + 1], None,
                                                op0=AL.mult)
                    nc.gpsimd.tensor_add(cvt[:, 0, :], cvt[:, 0, :],
                                         cvt[:, 1, :])
                    nc.gpsimd.tensor_add(cvs, cvs, cvt[:, 2, :])
                    nc.gpsimd.tensor_add(cvs, cvs, cvt[:, 0, :])
                for cp in ((0, 4), (4, 4), (8, 4), (12, 4), (16, 1)):
                    c0, n_in = cp
                    w = n_in * BLK
                    cvp = cv[:, c0:c0 + n_in, :]
                    th2 = pcm.tile([128, 4 * BLK], BF16, tag="th2")
                    nc.scalar.activation(th2[:, 0:w], cvp, AF.Tanh, scale=0.5)
                    nc.vector.tensor_scalar(th2[:, 0:w], th2[:, 0:w], 0.5, 0.5,
                                            op0=AL.mult, op1=AL.add)
                    nc.gpsimd.tensor_tensor(cvp, th2[:, 0:w], cvp, op=AL.mult)
                # cv[:, 0:16] = x (silu'd), cv[:, 16] = B (parts 0:64) | C
                ctc = psm.tile([NST, BLK], BF16, tag="ctc")
                ctc_store[b] = ctc
                nc.sync.dma_start(ctc[:], cv[NST:128, 16, :])

            sz_store = {}
            cv_store = {}
            ctc_store = {}

            prologue = prep_inproj_xbc(0)
            for fn in prologue:
                fn()
            emit_conv(0)
            emit_z(0)

            for b in range(NBLK):
                tsl = slice(b * BLK, (b + 1) * BLK)
                cv = cv_store.pop(b)
                ctc = ctc_store.pop(b)
                steal = iter(prep_inproj_xbc(b + 1) if b + 1 < NBLK else [])

                g_sb = pg.tile([128, ET, BLK], BF16)

                for qi in range(QPB):
                    qsl = slice(qi * Q, (qi + 1) * Q)
                    gsl = slice(b * BLK + qi * Q, b * BLK + (qi + 1) * Q)
                    cidx = b * QPB + qi
                    cum = pcm.tile([H, Q], FP32, tag="cum")
                    nc.vector.tensor_tensor_scan(cum[:], dtf[:, gsl], zeros32[:],
                                                 0.0, op0=AL.add, op1=AL.add)
                    ptr = ps_tr.tile([Q, H], FP32, tag="tr")
                    nc.tensor.transpose(ptr[:], cum[:], identf[0:H, 0:H])
                    ctall = pcm.tile([Q, H], FP32, tag="ctall")
                    nc.scalar.copy(ctall[:], ptr[:])
                    # exp bias per head: ln(dt_s) - cum_s  (dt folded into lt)
                    negcl = pcm.tile([Q, H], FP32, tag="negcl")
                    nc.vector.tensor_sub(negcl[:],
                                         lndttok[:, cidx * H:(cidx + 1) * H],
                                         ctall[:])

                    ptb = ps_tr.tile([Q, NST], BF16, tag="tr")
                    nc.tensor.transpose(ptb[:], cv[0:NST, 16, qsl],
                                        identb[0:NST, 0:NST])
                    btok = pcm.tile([Q, NST], BF16, tag="btok")
                    nc.scalar.copy(btok[:], ptb[:])

                    g0 = ps_tr.tile([Q, Q], FP32, tag="tr")
                    nc.tensor.matmul(g0[:], cv[0:NST, 16, qsl], ctc[:, qsl])
                    g0sb = pcm.tile([Q, Q], BF16, tag="g0sb")
                    nc.scalar.copy(g0sb[:], g0[:])

                    xtok = pxt.tile([Q, E], BF16, tag="xtok")
                    for f4 in range(ET // 4):
                        ptx = ps_tr.tile([Q, 4 * 128], BF16, tag="tr")
                        for j in range(4):
                            nc.tensor.transpose(
                                ptx[:, j * 128:(j + 1) * 128],
                                cv[:, 4 * f4 + j, qsl], identb[:])
                        nc.vector.tensor_copy(
                            xtok[:, f4 * 512:(f4 + 1) * 512], ptx[:])

                    for _ in range(2):
                        fn = next(steal, None)
                        if fn is not None:
                            fn()
                    for hg in range(H // 8):
                        h0 = hg * 8
                        bcq8 = pb2.tile([Q, 8 * Q], FP32, tag="bcq")
                        for hh in range(2):
                            stg = psc.tile([1, 4 * Q], FP32, tag="stg")
                            nc.sync.dma_start(
                                stg[:], cum[h0 + 4 * hh:h0 + 4 * hh + 4, :])
                            nc.gpsimd.partition_broadcast(
                                bcq8[:, 4 * hh * Q:4 * (hh + 1) * Q], stg[:])
                        bce8 = pb1.tile([Q, 8 * Q], FP32, tag="bce")
                        nc.scalar.activation(bce8[:], bcq8[:], AF.Exp)
                        # mask AFTER bce8 snapshot (WAR dep keeps order)
                        nc.gpsimd.tensor_add(bcq8[:], bcq8[:], mask8[:])
                        for kp in range(4):
                            ftp = (h0 + 2 * kp) // 2
                            yp2 = ps_yp.tile([128, Q], FP32, tag="yp")
                            for k2 in range(2):
                                k = 2 * kp + k2
                                h = h0 + k
                                ro = k2 * 64
                                csl = slice(h * HD, (h + 1) * HD)
                                ksl = slice(k * Q, (k + 1) * Q)
                                lt = psc.tile([Q, Q], FP32, tag="lt")
                                nc.scalar.activation(lt[:], bcq8[:, ksl],
                                                     AF.Exp,
                                                     bias=negcl[:, h:h + 1])
                                mt_t = psc.tile([Q, Q], BF16, tag="mt")
                                nc.gpsimd.tensor_mul(mt_t[:], g0sb[:], lt[:])
                                cpos = psc.tile([NST, Q], BF16, tag="cpos")
                                nc.gpsimd.tensor_mul(cpos[:], ctc[:, qsl],
                                                     bce8[0:NST, ksl])
                                bh = psc.tile([Q, NST], BF16, tag="bh")
                                nc.vector.tensor_scalar(
                                    bh[:], btok[:],
                                    lt[:, Q - 1:Q],
                                    None, op0=AL.mult)
                                nc.tensor.matmul(yp2[ro:ro + 64, :],
                                                 xtok[:, csl], mt_t[:],
                                                 start=True, stop=False)
                                nc.tensor.matmul(yp2[ro:ro + 64, :],
                                                 S[:, csl], cpos[:],
                                                 start=False, stop=True)
                                sp = ps_sp.tile([NST, HD], FP32, tag="sp")
                                nc.tensor.matmul(sp[:], bh[:], xtok[:, csl])
                                nc.vector.scalar_tensor_tensor(
                                    S[:, csl], S[:, csl],
                                    bce8[0:NST, (k + 1) * Q - 1:(k + 1) * Q],
                                    sp[:], op0=AL.mult, op1=AL.add)
                            nc.vector.scalar_tensor_tensor(
                                g_sb[:, ftp, qsl], cv[:, ftp, qsl],
                                dxt[:, ftp:ftp + 1], yp2[:],
                                op0=AL.mult, op1=AL.add)
                        # weave a couple of next-block in_proj pieces into
                        # the scan so PE/DVE gaps get filled
                        for _ in range(3):
                            fn = next(steal, None)
                            if fn is not None:
                                fn()
                for fn in steal:
                    fn()

                # ---- gating, sum of squares (scale deferred)
                sz = sz_store.pop(b)
                for gq in range(4):
                    nc.gpsimd.tensor_tensor(g_sb[:, 4 * gq:4 * gq + 4, :],
                                            g_sb[:, 4 * gq:4 * gq + 4, :],
                                            sz[:, 4 * gq:4 * gq + 4, :],
                                            op=AL.mult)
                ssq = ps_mm.tile([1, BLK], FP32, tag="mm")
                for ft in range(ET):
                    g2 = psm.tile([128, BLK], BF16, tag="th")
                    nc.vector.tensor_tensor(g2[:], g_sb[:, ft, :],
                                            g_sb[:, ft, :], op=AL.mult)
                    nc.tensor.matmul(ssq[:], onesb[:], g2[:],
                                     start=(ft == 0), stop=(ft == ET - 1))
                nc.vector.tensor_scalar(mall[0:1, tsl], ssq[:], 1.0 / E, EPS,
                                        op0=AL.mult, op1=AL.add)

                # conv of the next block runs on DVE/Pool while out_proj
                # owns the TensorEngine
                if b + 1 < NBLK:
                    emit_conv(b + 1)

                def mk_ho(mt):
                    def go():
                        ho = ps_mm.tile([128, BLK], FP32, tag="mm")
                        for kt in range(ET):
                            nc.tensor.matmul(
                                ho[:], w_out[:, kt, mt * 128:(mt + 1) * 128],
                                g_sb[:, kt, :],
                                start=(kt == 0), stop=(kt == ET - 1))
                        hob = psm.tile([128, BLK], BF16, tag="hob")
                        nc.vector.tensor_copy(hob[:], ho[:])
                        nc.sync.dma_start(ho_d[:, mt, tsl], hob[:])
                    return go
                ho_steps = iter([mk_ho(mt) for mt in range(ET // 2)])
                if b + 1 < NBLK:
                    emit_z(b + 1, ho_iter=ho_steps)
                for fn in ho_steps:
                    fn()

            # ---- layer epilogue: rmsnorm scale + selu (+ spill / pooling)
            nc.scalar.activation(mall[:], mall[:], AF.Ln)
            nc.scalar.activation(mall[:], mall[:], AF.Exp, scale=-0.5)
            for b in range(NBLK):
                tsl = slice(b * BLK, (b + 1) * BLK)
                scb = psm.tile([128, BLK], BF16, tag="scb")
                nc.gpsimd.partition_broadcast(scb[:], mall[0:1, tsl])
                for mt in range(ET // 2):
                    hot = pio.tile([128, BLK], BF16, tag="hot")
                    nc.sync.dma_start(hot[:], ho_d[:, mt, tsl])
                    nc.gpsimd.tensor_mul(hot[:], hot[:], scb[:])
                    rl = psm.tile([128, BLK], BF16, tag="rl")
                    nc.scalar.activation(rl[:], hot[:], AF.Relu, scale=SELU_L)
                    ex = psm.tile([128, BLK], BF16, tag="ex")
                    nc.scalar.activation(ex[:], hot[:], AF.Exp, bias=lnla_t[:])
                    nc.vector.tensor_scalar(ex[:], ex[:], SELU_LA, SELU_LA,
                                            op0=AL.min, op1=AL.subtract)
                    nc.gpsimd.tensor_tensor(rl[:], rl[:], ex[:], op=AL.add)
                    if layer == 0:
                        nc.sync.dma_start(u2[:, mt, tsl], rl[:])
                    else:
                        red = psm.tile([128, 1], FP32, tag="red")
                        nc.vector.tensor_reduce(red[:], rl[:],
                                                axis=mybir.AxisListType.X,
                                                op=AL.add)
                        nc.vector.tensor_add(pacc[:, mt:mt + 1],
                                             pacc[:, mt:mt + 1], red[:])

        pooled = psm.tile([128, KT_D], FP32, tag="pooled")
        nc.vector.tensor_scalar(pooled[:], pacc[:, 0:KT_D], 1.0 / L, None,
                                op0=AL.mult)
        ph = ps_sp.tile([1, 10], FP32, tag="sp")
        for kt in range(KT_D):
            nc.tensor.matmul(ph[:], pooled[:, kt:kt + 1], whead[:, kt, :],
                             start=(kt == 0), stop=(kt == KT_D - 1))
        ot = psm.tile([1, 10], FP32, tag="ot")
        nc.vector.tensor_add(ot[:], ph[:], bcat[:])
        nc.sync.dma_start(out_d[:], ot[:])

    nc.compile()
    return nc


def _host_inputs(inputs):
    m = {}
    m.update(_prep_layer(inputs, "1"))
    m.update(_prep_layer(inputs, "2"))
    j = np.arange(128)
    mneg = np.where(j[None, :] >= j[:, None], 0.0, -1e30)
    m["mask8"] = _bf(np.tile(mneg, (1, 8)))
    m["identb"] = _bf(np.eye(128))
    m["identf"] = _f32(np.eye(128))
    m["onesb"] = _bf(np.ones((128, 1)))
    wcat = np.concatenate([np.asarray(inputs["w_emo"], np.float32),
                           np.asarray(inputs["w_sen"], np.float32)], 0)
    m["whead"] = _f32(wcat.T.reshape(KT_D, 128, 10).transpose(1, 0, 2))
    m["bcat"] = _f32(np.concatenate([inputs["b_emo"], inputs["b_sen"]])
                     .reshape(1, 10))
    return m


def kernel(**inputs) -> np.ndarray:
    if "nc" not in _CACHE:
        _CACHE["nc"] = _build()
    nc = _CACHE["nc"]

    x = np.asarray(inputs["x"], np.float32)
    shared = _host_inputs(inputs)
    in_maps = []
    for s in range(NCORE):
        m = dict(shared)
        xts = x[s].T.reshape(KT_D, 128, L).transpose(1, 0, 2)
        m["xt"] = _bf(xts)
        in_maps.append(m)

    res = run_bass_kernel_spmd(nc, in_maps, core_ids=list(range(NCORE)))
    out = np.concatenate([r["out"] for r in res.results], 0)
    return out.astype(np.float32)


# revision 54
# speedup vs baseline: 1.7411x; 1.0396x over previous
"""Trainium2 Bass kernel for nn_AudioModelM1 (2x Mamba2 + selu + pool + heads).

Sharding: data-parallel over batch — 8 samples -> 8 NeuronCores, one sample per
core, no collectives.  Per-core layout is feature-major (features on SBUF
partitions, tokens on the free dim).  The selective scan uses the chunked
(quadratic-intra / recurrent-inter) Mamba2 formulation with Q=128 token chunks
so all heavy math runs on the TensorEngine.

Engine-balance notes (CoreSim cost model):
  - dt (softplus) is hoisted to a per-layer preamble and the RMSNorm scale +
    SELU are deferred to a per-layer epilogue so Ln never interleaves with the
    Exp/Tanh activations inside the block loop (act-table reloads are 1.3us).
  - silu(x) = x * (0.5*tanh(x/2) + 0.5): Tanh lives in the same activation
    table as Exp, so no table switches and no DVE reciprocal.
  - Small elementwise work in the scan inner loop runs on the Pool engine
    (flat 0.833 ns/elem, no access penalty); DVE ops keep all tensor operands
    packed bf16 in SBUF to hit the 2x/4x DVE perf modes.
  - Decay matrices are built per 4-head group: Pool stt folds (-cum_s + mask)
    so the Act exp runs batched over [Q, 4Q].
"""
import sys
sys.path.insert(0, "/opt/trn_rl_repo")

from contextlib import ExitStack

import numpy as np
import ml_dtypes

import concourse.bass as bass
import concourse.tile as tile
from concourse import bacc, mybir
from concourse.bass_utils import run_bass_kernel_spmd

FP32 = mybir.dt.float32
BF16 = mybir.dt.bfloat16
AL = mybir.AluOpType
AF = mybir.ActivationFunctionType

D = 1024
E = 2048
NST = 64
HD = 64
H = 32
DCONV = 4
CCH = E + 2 * NST             # 2176 conv channels (17 tiles)
F = 2 * E + 2 * NST + H       # 4256 in_proj rows
L = 2048
NCORE = 8

BLK = 256
NBLK = L // BLK
Q = 128
QPB = BLK // Q

KT_D = D // 128
MT_F = 34
CT = CCH // 128
ET = E // 128
HP = 4

SELU_L = 1.0507009873554805
SELU_A = 1.6732632423543772
SELU_LA = SELU_L * SELU_A
LN_LA = float(np.log(SELU_LA))
EPS = 1e-5

_CACHE = {}


def _bf(x):
    return np.ascontiguousarray(np.asarray(x, np.float32).astype(ml_dtypes.bfloat16))


def _f32(x):
    return np.ascontiguousarray(np.asarray(x, np.float32))


def _prep_layer(w, suf):
    in_w = np.asarray(w["in_proj_w" + suf], np.float32)
    out_w = np.asarray(w["out_proj_w" + suf], np.float32)
    norm_w = np.asarray(w["norm_w" + suf], np.float32)
    conv_w = np.asarray(w["conv_w" + suf], np.float32)
    conv_b = np.asarray(w["conv_b" + suf], np.float32)
    dt_b = np.asarray(w["dt_bias" + suf], np.float32)
    A_log = np.asarray(w["A_log" + suf], np.float32)
    Dp = np.asarray(w["D" + suf], np.float32)

    win = in_w.T.reshape(KT_D, 128, F).transpose(1, 0, 2)
    wo = (out_w * norm_w[None, :]).T
    wout = wo.reshape(ET, 128, D).transpose(1, 0, 2)
    cw = conv_w.reshape(CT, 128, DCONV).transpose(1, 0, 2)
    cb = conv_b.reshape(CT, 128).T
    dx = np.repeat(Dp, HD).reshape(ET, 128).T
    return {
        "win" + suf: _bf(win),
        "wout" + suf: _bf(wout),
        "cw" + suf: _f32(cw),
        "cb" + suf: _f32(cb),
        "dtb" + suf: _f32(dt_b.reshape(H, 1)),
        "A" + suf: _f32(-np.exp(A_log).reshape(H, 1)),
        "dx" + suf: _f32(dx),
    }


def _build():
    nc = bacc.Bacc("TRN2")
    dram = {}

    def din(name, shape, dt):
        dram[name] = nc.dram_tensor(name, list(shape), dt, kind="ExternalInput")
        return dram[name]

    xt = din("xt", (128, KT_D, L), BF16)
    for suf in ("1", "2"):
        din("win" + suf, (128, KT_D, F), BF16)
        din("wout" + suf, (128, ET, D), BF16)
        din("cw" + suf, (128, CT, DCONV), FP32)
        din("cb" + suf, (128, CT), FP32)
        din("dtb" + suf, (H, 1), FP32)
        din("A" + suf, (H, 1), FP32)
        din("dx" + suf, (128, ET), FP32)
    din("mask8", (128, 8 * Q), BF16)
    din("identb", (128, 128), BF16)
    din("identf", (128, 128), FP32)
    din("onesb", (128, 1), BF16)
    din("whead", (128, KT_D, 10), FP32)
    din("bcat", (1, 10), FP32)

    u2 = nc.dram_tensor("u2spill", [128, KT_D, L], BF16)
    ho_d = nc.dram_tensor("hospill", [128, KT_D, L], BF16)
    out_d = nc.dram_tensor("out", [1, 10], FP32, kind="ExternalOutput")

    with nc.allow_low_precision(reason="bf16 staging is intentional"), \
            tile.TileContext(nc) as tc, ExitStack() as ctx:
        pw = ctx.enter_context(tc.tile_pool(name="weights", bufs=1))
        pconst = ctx.enter_context(tc.tile_pool(name="consts", bufs=1))
        pio = ctx.enter_context(tc.tile_pool(name="io", bufs=2))
        pz = ctx.enter_context(tc.tile_pool(name="zsil", bufs=1))
        pxbc = ctx.enter_context(tc.tile_pool(name="xbcin", bufs=1))
        pxc = ctx.enter_context(tc.tile_pool(name="xconv", bufs=1))
        pg = ctx.enter_context(tc.tile_pool(name="gate", bufs=1))
        psc = ctx.enter_context(tc.tile_pool(name="scan", bufs=2))
        pxt = ctx.enter_context(tc.tile_pool(name="xtok", bufs=1))
        pcm = ctx.enter_context(tc.tile_pool(name="chunkmeta", bufs=1))
        pb1 = ctx.enter_context(tc.tile_pool(name="bcq1", bufs=2))
        pb2 = ctx.enter_context(tc.tile_pool(name="bcq2", bufs=2))
        psm = ctx.enter_context(tc.tile_pool(name="small", bufs=2))
        pstate = ctx.enter_context(tc.tile_pool(name="state", bufs=1))
        pdt = ctx.enter_context(tc.tile_pool(name="dtpre", bufs=1))
        pcv1 = ctx.enter_context(tc.tile_pool(name="cv1", bufs=1))

        ps_mm = ctx.enter_context(tc.tile_pool(name="psmm", bufs=2, space="PSUM"))
        ps_tr = ctx.enter_context(tc.tile_pool(name="pstr", bufs=2, space="PSUM"))
        ps_yp = ctx.enter_context(tc.tile_pool(name="psyp", bufs=2, space="PSUM"))
        ps_sp = ctx.enter_context(tc.tile_pool(name="pssp", bufs=2, space="PSUM"))

        w_in = pw.tile([128, KT_D, F], BF16)
        w_out = pw.tile([128, ET, D], BF16)
        cw = pw.tile([128, CT, DCONV], FP32)
        cb = pw.tile([128, CT], FP32)
        dtb = pw.tile([H, 1], FP32)
        Atile = pw.tile([H, 1], FP32)
        dxt = pw.tile([128, ET], FP32)

        mask8 = pconst.tile([128, 8 * Q], BF16)
        identb = pconst.tile([128, 128], BF16)
        identf = pconst.tile([3 * H, 3 * H], FP32)
        onesb = pconst.tile([128, 1], BF16)
        whead = pconst.tile([128, KT_D, 10], FP32)
        bcat = pconst.tile([1, 10], FP32)
        zeros32 = pconst.tile([H, Q], FP32)
        ones32 = pconst.tile([H, 1], FP32)
        lnla_t = pconst.tile([128, 1], FP32)

        S = pstate.tile([NST, H * HD], BF16)   # [n, (h,p)] heads at base part 0
        pacc = pstate.tile([128, ET], FP32)

        # per-layer hoisted dt: raw -> exp -> (ln in place) -> dt; then alog
        dtf = pdt.tile([H, L], FP32)     # exp(raw+b) -> dt (f32) -> alog=dt*A
        lndttok = pdt.tile([Q, NBLK * QPB * H], FP32)  # token-major ln(dt)
        mall = pdt.tile([1, L], BF16)    # ssq/E+eps -> ln -> rmsnorm scale
        tailt = pdt.tile([128, CT, HP - 1], BF16)  # conv tail carry

        for t, name in ((mask8, "mask8"), (identb, "identb"),
                        (onesb, "onesb"), (whead, "whead"), (bcat, "bcat")):
            nc.sync.dma_start(t[:], dram[name][:])
        nc.sync.dma_start(identf[:], dram["identf"][0:3 * H, 0:3 * H])
        nc.vector.memset(zeros32[:], 0.0)
        nc.vector.memset(pacc[:], 0.0)
        nc.vector.memset(ones32[:], 1.0)
        nc.vector.memset(lnla_t[:], LN_LA)

        for layer in (0, 1):
            suf = "12"[layer]
            for t, name in ((w_in, "win"), (w_out, "wout"), (cw, "cw"), (cb, "cb"),
                            (dtb, "dtb"), (Atile, "A"), (dxt, "dx")):
                nc.sync.dma_start(t[:], dram[name + suf][:])
            nc.vector.memset(S[:], 0.0)

            src = xt if layer == 0 else u2

            # ---- dt preamble: dt_raw for the whole layer, one softplus ----
            for b in range(NBLK):
                tsl = slice(b * BLK, (b + 1) * BLK)
                u_t = pio.tile([128, KT_D, BLK], BF16, tag="ut")
                nc.sync.dma_start(u_t[:], src[:, :, tsl])
                pmmd = ps_mm.tile([H, BLK], FP32, tag="mm")
                for kt in range(KT_D):
                    nc.tensor.matmul(pmmd[:], w_in[:, kt, F - H:F],
                                     u_t[:, kt, :], start=(kt == 0),
                                     stop=(kt == KT_D - 1))
                nc.scalar.activation(dtf[:, tsl], pmmd[:], AF.Exp, bias=dtb[:])
            # softplus ln over the whole layer at once
            nc.scalar.activation(dtf[:], dtf[:], AF.Ln, bias=ones32[:])
            # token-major ln(dt) columns (still inside the Ln table window):
            # per-head decay matrices get dt folded in via the exp bias
            for c in range(NBLK * QPB):
                ptd = ps_tr.tile([Q, H], FP32, tag="tr")
                nc.tensor.transpose(ptd[:], dtf[:, c * Q:(c + 1) * Q],
                                    identf[0:H, 0:H])
                nc.scalar.activation(lndttok[:, c * H:(c + 1) * H], ptd[:],
                                     AF.Ln)
            nc.vector.tensor_scalar(dtf[:], dtf[:], Atile[:], None, op0=AL.mult)
            # dtf now holds alog = dt * A

            xbc = pxbc.tile([128, CT, BLK + HP], BF16)
            nc.vector.memset(xbc[:, :, 0:HP], 0.0)

            ut_store = {}

            def prep_inproj_xbc(b):
                """DMA u_t(b) now; return emit-closures for the xbc half of
                in_proj(b) to be woven into the previous block's scan."""
                u_t = pio.tile([128, KT_D, BLK], BF16, tag="ut")
                ut_store[b] = u_t
                nc.sync.dma_start(u_t[:], src[:, :, b * BLK:(b + 1) * BLK])
                steps = []
                if b > 0:
                    steps.append(lambda: nc.vector.tensor_copy(
                        tailt[:], xbc[:, :, BLK + 1:BLK + HP]))

                def mk(mt):
                    def go():
                        pmm = ps_mm.tile([128, BLK], FP32, tag="mm")
                        for kt in range(KT_D):
                            nc.tensor.matmul(
                                pmm[:], w_in[:, kt, mt * 128:(mt + 1) * 128],
                                u_t[:, kt, :], start=(kt == 0),
                                stop=(kt == KT_D - 1))
                        nc.vector.tensor_copy(xbc[:, mt - ET, HP:HP + BLK],
                                              pmm[:])
                    return go
                for mt in range(ET, 33):
                    steps.append(mk(mt))
                if b > 0:
                    steps.append(lambda: nc.vector.tensor_copy(
                        xbc[:, :, 1:HP], tailt[:]))
                return steps

            def emit_z(b, ho_iter=None):
                """z half of in_proj(b) -> sz; optionally interleave the
                previous block's out_proj groups from ho_iter."""
                u_t = ut_store.pop(b)
                sz = pz.tile([128, ET, BLK], BF16)
                sz_store[b] = sz
                for mt in range(ET):
                    if ho_iter is not None:
                        nxt = next(ho_iter, None)
                        if nxt is not None:
                            nxt()
                    pmm = ps_mm.tile([128, BLK], FP32, tag="mm")
                    for kt in range(KT_D):
                        nc.tensor.matmul(
                            pmm[:], w_in[:, kt, mt * 128:(mt + 1) * 128],
                            u_t[:, kt, :], start=(kt == 0), stop=(kt == KT_D - 1))
                    # silu(z) = z * (0.5*tanh(z/2) + 0.5)
                    th = psm.tile([128, BLK], BF16, tag="th")
                    nc.scalar.activation(th[:], pmm[:], AF.Tanh, scale=0.5)
                    zb = psm.tile([128, BLK], BF16, tag="zb")
                    nc.scalar.copy(zb[:], pmm[:])
                    nc.vector.tensor_scalar(th[:], th[:], 0.5, 0.5,
                                            op0=AL.mult, op1=AL.add)
                    nc.gpsimd.tensor_mul(sz[:, mt, :], th[:], zb[:])

            def emit_conv(b):
                # causal depthwise conv (+bias): 4 DVE tensor-scalar products
                # (4x perf mode) + 3 Pool tensor-tensor adds, then tanh-silu
                cv = pxc.tile([128, CT, BLK], BF16)
                cv_store[b] = cv
                for ct in range(CT):
                    cvs = cv[:, ct, :]
                    cvt = pcv1.tile([128, 3, BLK], BF16, tag="cvt")
                    nc.vector.tensor_scalar(cvs, xbc[:, ct, 1:1 + BLK],
                                            cw[:, ct, 0:1], cb[:, ct:ct + 1],
                                            op0=AL.mult, op1=AL.add)
                    for k in range(1, DCONV):
                        nc.vector.tensor_scalar(cvt[:, k - 1, :],
                                                xbc[:, ct, 1 + k:1 + k + BLK],
                                                cw[:, ct, k:k + 1], None,
                                                op0=AL.mult)
                    nc.gpsimd.tensor_add(cvt[:, 0, :], cvt[:, 0, :],
                                         cvt[:, 1, :])
                    nc.gpsimd.tensor_add(cvs, cvs, cvt[:, 2, :])
                    nc.gpsimd.tensor_add(cvs, cvs, cvt[:, 0, :])
                for cp in ((0, 4), (4, 4), (8, 4), (12, 4), (16, 1)):
                    c0, n_in = cp
                    w = n_in * BLK
                    cvp = cv[:, c0:c0 + n_in, :]
                    th2 = pcm.tile([128, 4 * BLK], BF16, tag="th2")
                    nc.scalar.activation(th2[:, 0:w], cvp, AF.Tanh, scale=0.5)
                    nc.vector.tensor_scalar(th2[:, 0:w], th2[:, 0:w], 0.5, 0.5,
                                            op0=AL.mult, op1=AL.add)
                    nc.gpsimd.tensor_tensor(cvp, th2[:, 0:w], cvp, op=AL.mult)
                # cv[:, 0:16] = x (silu'd), cv[:, 16] = B (parts 0:64) | C
                ctc = psm.tile([NST, BLK], BF16, tag="ctc")
                ctc_store[b] = ctc
                nc.sync.dma_start(ctc[:], cv[NST:128, 16, :])

            sz_store = {}
            cv_store = {}
            ctc_store = {}

            prologue = prep_inproj_xbc(0)
            for fn in prologue:
                fn()
            emit_conv(0)
            emit_z(0)

            for b in range(NBLK):
                tsl = slice(b * BLK, (b + 1) * BLK)
                cv = cv_store.pop(b)
                ctc = ctc_store.pop(b)
                steal = iter(prep_inproj_xbc(b + 1) if b + 1 < NBLK else [])

                g_sb = pg.tile([128, ET, BLK], BF16)

                for qi in range(QPB):
                    qsl = slice(qi * Q, (qi + 1) * Q)
                    gsl = slice(b * BLK + qi * Q, b * BLK + (qi + 1) * Q)
                    cidx = b * QPB + qi
                    cum = pcm.tile([H, Q], FP32, tag="cum")
                    nc.vector.tensor_tensor_scan(cum[:], dtf[:, gsl], zeros32[:],
                                                 0.0, op0=AL.add, op1=AL.add)
                    ptr = ps_tr.tile([Q, H], FP32, tag="tr")
                    nc.tensor.transpose(ptr[:], cum[:], identf[0:H, 0:H])
                    ctall = pcm.tile([Q, H], FP32, tag="ctall")
                    nc.scalar.copy(ctall[:], ptr[:])
                    # exp bias per head: ln(dt_s) - cum_s  (dt folded into lt)
                    negcl = pcm.tile([Q, H], FP32, tag="negcl")
                    nc.vector.tensor_sub(negcl[:],
                                         lndttok[:, cidx * H:(cidx + 1) * H],
                                         ctall[:])

                    ptb = ps_tr.tile([Q, NST], BF16, tag="tr")
                    nc.tensor.transpose(ptb[:], cv[0:NST, 16, qsl],
                                        identb[0:NST, 0:NST])
                    btok = pcm.tile([Q, NST], BF16, tag="btok")
                    nc.scalar.copy(btok[:], ptb[:])

                    g0 = ps_tr.tile([Q, Q], FP32, tag="tr")
                    nc.tensor.matmul(g0[:], cv[0:NST, 16, qsl], ctc[:, qsl])
                    g0sb = pcm.tile([Q, Q], BF16, tag="g0sb")
                    nc.scalar.copy(g0sb[:], g0[:])

                    xtok = pxt.tile([Q, E], BF16, tag="xtok")
                    for f4 in range(ET // 4):
                        ptx = ps_tr.tile([Q, 4 * 128], BF16, tag="tr")
                        for j in range(4):
                            nc.tensor.transpose(
                                ptx[:, j * 128:(j + 1) * 128],
                                cv[:, 4 * f4 + j, qsl], identb[:])
                        nc.vector.tensor_copy(
                            xtok[:, f4 * 512:(f4 + 1) * 512], ptx[:])

                    for _ in range(2):
                        fn = next(steal, None)
                        if fn is not None:
                            fn()
                    for hg in range(H // 8):
                        h0 = hg * 8
                        bcq8 = pb2.tile([Q, 8 * Q], FP32, tag="bcq")
                        for hh in range(2):
                            stg = psc.tile([1, 4 * Q], FP32, tag="stg")
                            nc.sync.dma_start(
                                stg[:], cum[h0 + 4 * hh:h0 + 4 * hh + 4, :])
                            nc.gpsimd.partition_broadcast(
                                bcq8[:, 4 * hh * Q:4 * (hh + 1) * Q], stg[:])
                        bce8 = pb1.tile([Q, 8 * Q], FP32, tag="bce")
                        nc.scalar.activation(bce8[:], bcq8[:], AF.Exp)
                        # mask AFTER bce8 snapshot (WAR dep keeps order)
                        nc.gpsimd.tensor_add(bcq8[:], bcq8[:], mask8[:])
                        for kp in range(4):
                            ftp = (h0 + 2 * kp) // 2
                            yp2 = ps_yp.tile([128, Q], FP32, tag="yp")
                            for k2 in range(2):
                                k = 2 * kp + k2
                                h = h0 + k
                                ro = k2 * 64
                                csl = slice(h * HD, (h + 1) * HD)
                                ksl = slice(k * Q, (k + 1) * Q)
                                lt = psc.tile([Q, Q], FP32, tag="lt")
                                nc.scalar.activation(lt[:], bcq8[:, ksl],
                                                     AF.Exp,
                                                     bias=negcl[:, h:h + 1])
                                mt_t = psc.tile([Q, Q], BF16, tag="mt")
                                nc.gpsimd.tensor_mul(mt_t[:], g0sb[:], lt[:])
                                cpos = psc.tile([NST, Q], BF16, tag="cpos")
                                nc.gpsimd.tensor_mul(cpos[:], ctc[:, qsl],
                                                     bce8[0:NST, ksl])
                                bh = psc.tile([Q, NST], BF16, tag="bh")
                                nc.vector.tensor_scalar(
                                    bh[:], btok[:],
                                    lt[:, Q - 1:Q],
                                    None, op0=AL.mult)
                                nc.tensor.matmul(yp2[ro:ro + 64, :],
                                                 xtok[:, csl], mt_t[:],
                                                 start=True, stop=False)
                                nc.tensor.matmul(yp2[ro:ro + 64, :],
                                                 S[:, csl], cpos[:],
                                                 start=False, stop=True)
                                sp = ps_sp.tile([NST, HD], FP32, tag="sp")
                                nc.tensor.matmul(sp[:], bh[:], xtok[:, csl])
                                nc.vector.scalar_tensor_tensor(
                                    S[:, csl], S[:, csl],
                                    bce8[0:NST, (k + 1) * Q - 1:(k + 1) * Q],
                                    sp[:], op0=AL.mult, op1=AL.add)
                            nc.vector.scalar_tensor_tensor(
                                g_sb[:, ftp, qsl], cv[:, ftp, qsl],
                                dxt[:, ftp:ftp + 1], yp2[:],
                                op0=AL.mult, op1=AL.add)
                        # weave a couple of next-block in_proj pieces into
                        # the scan so PE/DVE gaps get filled
                        for _ in range(2):
                            fn = next(steal, None)
                            if fn is not None:
                                fn()
                for fn in steal:
                    fn()

                # ---- gating, sum of squares (scale deferred)
                sz = sz_store.pop(b)
                for gq in range(4):
                    nc.gpsimd.tensor_tensor(g_sb[:, 4 * gq:4 * gq + 4, :],
                                            g_sb[:, 4 * gq:4 * gq + 4, :],
                                            sz[:, 4 * gq:4 * gq + 4, :],
                                            op=AL.mult)
                ssq = ps_mm.tile([1, BLK], FP32, tag="mm")
                for ft in range(ET):
                    g2 = psm.tile([128, BLK], BF16, tag="th")
                    nc.vector.tensor_tensor(g2[:], g_sb[:, ft, :],
                                            g_sb[:, ft, :], op=AL.mult)
                    nc.tensor.matmul(ssq[:], onesb[:], g2[:],
                                     start=(ft == 0), stop=(ft == ET - 1))
                nc.vector.tensor_scalar(mall[0:1, tsl], ssq[:], 1.0 / E, EPS,
                                        op0=AL.mult, op1=AL.add)

                # conv of the next block runs on DVE/Pool while out_proj
                # owns the TensorEngine
                if b + 1 < NBLK:
                    emit_conv(b + 1)

                def mk_ho(mt):
                    def go():
                        ho = ps_mm.tile([128, BLK], FP32, tag="mm")
                        for kt in range(ET):
                            nc.tensor.matmul(
                                ho[:], w_out[:, kt, mt * 128:(mt + 1) * 128],
                                g_sb[:, kt, :],
                                start=(kt == 0), stop=(kt == ET - 1))
                        hob = psm.tile([128, BLK], BF16, tag="hob")
                        nc.vector.tensor_copy(hob[:], ho[:])
                        nc.sync.dma_start(ho_d[:, mt, tsl], hob[:])
                    return go
                ho_steps = iter([mk_ho(mt) for mt in range(ET // 2)])
                if b + 1 < NBLK:
                    emit_z(b + 1, ho_iter=ho_steps)
                for fn in ho_steps:
                    fn()

            # ---- layer epilogue: rmsnorm scale + selu (+ spill / pooling)
            nc.scalar.activation(mall[:], mall[:], AF.Ln)
            nc.scalar.activation(mall[:], mall[:], AF.Exp, scale=-0.5)
            for b in range(NBLK):
                tsl = slice(b * BLK, (b + 1) * BLK)
                scb = psm.tile([128, BLK], BF16, tag="scb")
                nc.gpsimd.partition_broadcast(scb[:], mall[0:1, tsl])
                for mt in range(ET // 2):
                    hot = pio.tile([128, BLK], BF16, tag="hot")
                    nc.sync.dma_start(hot[:], ho_d[:, mt, tsl])
                    nc.gpsimd.tensor_mul(hot[:], hot[:], scb[:])
                    rl = psm.tile([128, BLK], BF16, tag="rl")
                    nc.scalar.activation(rl[:], hot[:], AF.Relu, scale=SELU_L)
                    ex = psm.tile([128, BLK], BF16, tag="ex")
                    nc.scalar.activation(ex[:], hot[:], AF.Exp, bias=lnla_t[:])
                    nc.vector.tensor_scalar(ex[:], ex[:], SELU_LA, SELU_LA,
                                            op0=AL.min, op1=AL.subtract)
                    nc.gpsimd.tensor_tensor(rl[:], rl[:], ex[:], op=AL.add)
                    if layer == 0:
                        nc.sync.dma_start(u2[:, mt, tsl], rl[:])
                    else:
                        red = psm.tile([128, 1], FP32, tag="red")
                        nc.vector.tensor_reduce(red[:], rl[:],
                                                axis=mybir.AxisListType.X,
                                                op=AL.add)
                        nc.vector.tensor_add(pacc[:, mt:mt + 1],
                                             pacc[:, mt:mt + 1], red[:])

        pooled = psm.tile([128, KT_D], FP32, tag="pooled")
        nc.vector.tensor_scalar(pooled[:], pacc[:, 0:KT_D], 1.0 / L, None,
                                op0=AL.mult)
        ph = ps_sp.tile([1, 10], FP32, tag="sp")
        for kt in range(KT_D):
            nc.tensor.matmul(ph[:], pooled[:, kt:kt + 1], whead[:, kt, :],
                             start=(kt == 0), stop=(kt == KT_D - 1))
        ot = psm.tile([1, 10], FP32, tag="ot")
        nc.vector.tensor_add(ot[:], ph[:], bcat[:])
        nc.sync.dma_start(out_d[:], ot[:])

    nc.compile()
    return nc


def _host_inputs(inputs):
    m = {}
    m.update(_prep_layer(inputs, "1"))
    m.update(_prep_layer(inputs, "2"))
    j = np.arange(128)
    mneg = np.where(j[None, :] >= j[:, None], 0.0, -1e30)
    m["mask8"] = _bf(np.tile(mneg, (1, 8)))
    m["identb"] = _bf(np.eye(128))
    m["identf"] = _f32(np.eye(128))
    m["onesb"] = _bf(np.ones((128, 1)))
    wcat = np.concatenate([np.asarray(inputs["w_emo"], np.float32),
                           np.asarray(inputs["w_sen"], np.float32)], 0)
    m["whead"] = _f32(wcat.T.reshape(KT_D, 128, 10).transpose(1, 0, 2))
    m["bcat"] = _f32(np.concatenate([inputs["b_emo"], inputs["b_sen"]])
                     .reshape(1, 10))
    return m


def kernel(**inputs) -> np.ndarray:
    if "nc" not in _CACHE:
        _CACHE["nc"] = _build()
    nc = _CACHE["nc"]

    x = np.asarray(inputs["x"], np.float32)
    shared = _host_inputs(inputs)
    in_maps = []
    for s in range(NCORE):
        m = dict(shared)
        xts = x[s].T.reshape(KT_D, 128, L).transpose(1, 0, 2)
        m["xt"] = _bf(xts)
        in_maps.append(m)

    res = run_bass_kernel_spmd(nc, in_maps, core_ids=list(range(NCORE)))
    out = np.concatenate([r["out"] for r in res.results], 0)
    return out.astype(np.float32)


# revision 60
# speedup vs baseline: 1.7778x; 1.0211x over previous
"""Trainium2 Bass kernel for nn_AudioModelM1 (2x Mamba2 + selu + pool + heads).

Sharding: data-parallel over batch — 8 samples -> 8 NeuronCores, one sample per
core, no collectives.  Per-core layout is feature-major (features on SBUF
partitions, tokens on the free dim).  The selective scan uses the chunked
(quadratic-intra / recurrent-inter) Mamba2 formulation with Q=128 token chunks
so all heavy math runs on the TensorEngine.

Engine-balance notes (CoreSim cost model):
  - dt (softplus) is hoisted to a per-layer preamble and the RMSNorm scale +
    SELU are deferred to a per-layer epilogue so Ln never interleaves with the
    Exp/Tanh activations inside the block loop (act-table reloads are 1.3us).
  - silu(x) = x * (0.5*tanh(x/2) + 0.5): Tanh lives in the same activation
    table as Exp, so no table switches and no DVE reciprocal.
  - Small elementwise work in the scan inner loop runs on the Pool engine
    (flat 0.833 ns/elem, no access penalty); DVE ops keep all tensor operands
    packed bf16 in SBUF to hit the 2x/4x DVE perf modes.
  - Decay matrices are built per 4-head group: Pool stt folds (-cum_s + mask)
    so the Act exp runs batched over [Q, 4Q].
"""
import sys
sys.path.insert(0, "/opt/trn_rl_repo")

from contextlib import ExitStack

import numpy as np
import ml_dtypes

import concourse.bass as bass
import concourse.tile as tile
from concourse import bacc, mybir
from concourse.bass_utils import run_bass_kernel_spmd

FP32 = mybir.dt.float32
BF16 = mybir.dt.bfloat16
AL = mybir.AluOpType
AF = mybir.ActivationFunctionType

D = 1024
E = 2048
NST = 64
HD = 64
H = 32
DCONV = 4
CCH = E + 2 * NST             # 2176 conv channels (17 tiles)
F = 2 * E + 2 * NST + H       # 4256 in_proj rows
L = 2048
NCORE = 8

BLK = 256
NBLK = L // BLK
Q = 128
QPB = BLK // Q

KT_D = D // 128
MT_F = 34
CT = CCH // 128
ET = E // 128
HP = 4

SELU_L = 1.0507009873554805
SELU_A = 1.6732632423543772
SELU_LA = SELU_L * SELU_A
LN_LA = float(np.log(SELU_LA))
EPS = 1e-5

_CACHE = {}


def _bf(x):
    return np.ascontiguousarray(np.asarray(x, np.float32).astype(ml_dtypes.bfloat16))


def _f32(x):
    return np.ascontiguousarray(np.asarray(x, np.float32))


def _prep_layer(w, suf):
    in_w = np.asarray(w["in_proj_w" + suf], np.float32)
    out_w = np.asarray(w["out_proj_w" + suf], np.float32)
    norm_w = np.asarray(w["norm_w" + suf], np.float32)
    conv_w = np.asarray(w["conv_w" + suf], np.float32)
    conv_b = np.asarray(w["conv_b" + suf], np.float32)
    dt_b = np.asarray(w["dt_bias" + suf], np.float32)
    A_log = np.asarray(w["A_log" + suf], np.float32)
    Dp = np.asarray(w["D" + suf], np.float32)

    win = in_w.T.reshape(KT_D, 128, F).transpose(1, 0, 2)
    wo = (out_w * norm_w[None, :]).T
    wout = wo.reshape(ET, 128, D).transpose(1, 0, 2)
    cw = conv_w.reshape(CT, 128, DCONV).transpose(1, 0, 2)
    cb = conv_b.reshape(CT, 128).T
    dx = np.repeat(Dp, HD).reshape(ET, 128).T
    return {
        "win" + suf: _bf(win),
        "wout" + suf: _bf(wout),
        "cw" + suf: _f32(cw),
        "cb" + suf: _f32(cb),
        "dtb" + suf: _f32(dt_b.reshape(H, 1)),
        "A" + suf: _f32(-np.exp(A_log).reshape(H, 1)),
        "dx" + suf: _f32(dx),
    }


def _build():
    nc = bacc.Bacc("TRN2")
    dram = {}

    def din(name, shape, dt):
        dram[name] = nc.dram_tensor(name, list(shape), dt, kind="ExternalInput")
        return dram[name]

    xt = din("xt", (128, KT_D, L), BF16)
    for suf in ("1", "2"):
        din("win" + suf, (128, KT_D, F), BF16)
        din("wout" + suf, (128, ET, D), BF16)
        din("cw" + suf, (128, CT, DCONV), FP32)
        din("cb" + suf, (128, CT), FP32)
        din("dtb" + suf, (H, 1), FP32)
        din("A" + suf, (H, 1), FP32)
        din("dx" + suf, (128, ET), FP32)
    din("mask8", (128, 8 * Q), BF16)
    din("identb", (128, 128), BF16)
    din("identf", (128, 128), FP32)
    din("onesb", (128, 1), BF16)
    din("whead", (128, KT_D, 10), FP32)
    din("bcat", (1, 10), FP32)

    u2 = nc.dram_tensor("u2spill", [128, KT_D, L], BF16)
    ho_d = nc.dram_tensor("hospill", [128, KT_D, L], BF16)
    out_d = nc.dram_tensor("out", [1, 10], FP32, kind="ExternalOutput")

    with nc.allow_low_precision(reason="bf16 staging is intentional"), \
            tile.TileContext(nc) as tc, ExitStack() as ctx:
        pw = ctx.enter_context(tc.tile_pool(name="weights", bufs=1))
        pconst = ctx.enter_context(tc.tile_pool(name="consts", bufs=1))
        pio = ctx.enter_context(tc.tile_pool(name="io", bufs=2))
        pz = ctx.enter_context(tc.tile_pool(name="zsil", bufs=1))
        pxbc = ctx.enter_context(tc.tile_pool(name="xbcin", bufs=1))
        pxc = ctx.enter_context(tc.tile_pool(name="xconv", bufs=1))
        pg = ctx.enter_context(tc.tile_pool(name="gate", bufs=1))
        psc = ctx.enter_context(tc.tile_pool(name="scan", bufs=2))
        pxt = ctx.enter_context(tc.tile_pool(name="xtok", bufs=1))
        pcm = ctx.enter_context(tc.tile_pool(name="chunkmeta", bufs=1))
        pb1 = ctx.enter_context(tc.tile_pool(name="bcq1", bufs=2))
        pb2 = ctx.enter_context(tc.tile_pool(name="bcq2", bufs=2))
        psm = ctx.enter_context(tc.tile_pool(name="small", bufs=2))
        pstate = ctx.enter_context(tc.tile_pool(name="state", bufs=1))
        pdt = ctx.enter_context(tc.tile_pool(name="dtpre", bufs=1))
        pcv1 = ctx.enter_context(tc.tile_pool(name="cv1", bufs=1))

        ps_mm = ctx.enter_context(tc.tile_pool(name="psmm", bufs=2, space="PSUM"))
        ps_tr = ctx.enter_context(tc.tile_pool(name="pstr", bufs=2, space="PSUM"))
        ps_yp = ctx.enter_context(tc.tile_pool(name="psyp", bufs=2, space="PSUM"))
        ps_sp = ctx.enter_context(tc.tile_pool(name="pssp", bufs=2, space="PSUM"))

        w_in = pw.tile([128, KT_D, F], BF16)
        w_out = pw.tile([128, ET, D], BF16)
        cw = pw.tile([128, CT, DCONV], FP32)
        cb = pw.tile([128, CT], FP32)
        dtb = pw.tile([H, 1], FP32)
        Atile = pw.tile([H, 1], FP32)
        dxt = pw.tile([128, ET], FP32)

        mask8 = pconst.tile([128, 8 * Q], BF16)
        identb = pconst.tile([128, 128], BF16)
        identf = pconst.tile([3 * H, 3 * H], FP32)
        onesb = pconst.tile([128, 1], BF16)
        whead = pconst.tile([128, KT_D, 10], FP32)
        bcat = pconst.tile([1, 10], FP32)
        zeros32 = pconst.tile([H, Q], FP32)
        ones32 = pconst.tile([H, 1], FP32)
        lnla_t = pconst.tile([128, 1], FP32)
        wdt2 = pconst.tile([128, KT_D, H], BF16)
        dtb2c = pconst.tile([H, 1], FP32)

        S = pstate.tile([NST, H * HD], BF16)   # [n, (h,p)] heads at base part 0
        pacc = pstate.tile([128, ET], FP32)

        # per-layer hoisted dt: raw -> exp -> (ln in place) -> dt; then alog
        dtf = pdt.tile([H, L], FP32)     # exp(raw+b) -> dt (f32) -> alog=dt*A
        lndttok = pdt.tile([Q, NBLK * QPB * H], FP32)  # token-major ln(dt)
        mall = pdt.tile([1, L], BF16)    # ssq/E+eps -> ln -> rmsnorm scale
        tailt = pdt.tile([128, CT, HP - 1], BF16)  # conv tail carry

        for t, name in ((mask8, "mask8"), (identb, "identb"),
                        (onesb, "onesb"), (whead, "whead"), (bcat, "bcat")):
            nc.sync.dma_start(t[:], dram[name][:])
        nc.sync.dma_start(identf[:], dram["identf"][0:3 * H, 0:3 * H])
        nc.sync.dma_start(wdt2[:], dram["win2"][:, :, F - H:F])
        nc.sync.dma_start(dtb2c[:], dram["dtb2"][:])
        nc.vector.memset(zeros32[:], 0.0)
        nc.vector.memset(pacc[:], 0.0)
        nc.vector.memset(ones32[:], 1.0)
        nc.vector.memset(lnla_t[:], LN_LA)

        for layer in (0, 1):
            suf = "12"[layer]
            for t, name in ((w_in, "win"), (w_out, "wout"), (cw, "cw"), (cb, "cb"),
                            (dtb, "dtb"), (Atile, "A"), (dxt, "dx")):
                nc.sync.dma_start(t[:], dram[name + suf][:])
            nc.vector.memset(S[:], 0.0)

            src = xt if layer == 0 else u2

            # ---- dt preamble: dt_raw for the whole layer, one softplus.
            # Layer 2's exp(raw+bias) is produced by the fused layer-1
            # epilogue, so only layer 1 runs the matmul sweep here.
            if layer == 0:
                for b in range(NBLK):
                    tsl = slice(b * BLK, (b + 1) * BLK)
                    u_t = pio.tile([128, KT_D, BLK], BF16, tag="ut")
                    nc.sync.dma_start(u_t[:], src[:, :, tsl])
                    pmmd = ps_mm.tile([H, BLK], FP32, tag="mm")
                    for kt in range(KT_D):
                        nc.tensor.matmul(pmmd[:], w_in[:, kt, F - H:F],
                                         u_t[:, kt, :], start=(kt == 0),
                                         stop=(kt == KT_D - 1))
                    nc.scalar.activation(dtf[:, tsl], pmmd[:], AF.Exp,
                                         bias=dtb[:])
            # softplus ln over the whole layer at once
            nc.scalar.activation(dtf[:], dtf[:], AF.Ln, bias=ones32[:])
            # token-major ln(dt) columns (still inside the Ln table window):
            # per-head decay matrices get dt folded in via the exp bias
            for c in range(NBLK * QPB):
                ptd = ps_tr.tile([Q, H], FP32, tag="tr")
                nc.tensor.transpose(ptd[:], dtf[:, c * Q:(c + 1) * Q],
                                    identf[0:H, 0:H])
                nc.scalar.activation(lndttok[:, c * H:(c + 1) * H], ptd[:],
                                     AF.Ln)
            nc.vector.tensor_scalar(dtf[:], dtf[:], Atile[:], None, op0=AL.mult)
            # dtf now holds alog = dt * A

            xbc = pxbc.tile([128, CT, BLK + HP], BF16)
            nc.vector.memset(xbc[:, :, 0:HP], 0.0)

            ut_store = {}

            def prep_inproj_xbc(b):
                """DMA u_t(b) now; return emit-closures for the xbc half of
                in_proj(b) to be woven into the previous block's scan."""
                u_t = pio.tile([128, KT_D, BLK], BF16, tag="ut")
                ut_store[b] = u_t
                nc.sync.dma_start(u_t[:], src[:, :, b * BLK:(b + 1) * BLK])
                steps = []
                if b > 0:
                    steps.append(lambda: nc.vector.tensor_copy(
                        tailt[:], xbc[:, :, BLK + 1:BLK + HP]))

                def mk(mt):
                    def go():
                        pmm = ps_mm.tile([128, BLK], FP32, tag="mm")
                        for kt in range(KT_D):
                            nc.tensor.matmul(
                                pmm[:], w_in[:, kt, mt * 128:(mt + 1) * 128],
                                u_t[:, kt, :], start=(kt == 0),
                                stop=(kt == KT_D - 1))
                        nc.vector.tensor_copy(xbc[:, mt - ET, HP:HP + BLK],
                                              pmm[:])
                    return go
                for mt in range(ET, 33):
                    steps.append(mk(mt))
                if b > 0:
                    steps.append(lambda: nc.vector.tensor_copy(
                        xbc[:, :, 1:HP], tailt[:]))
                return steps

            def emit_z(b, ho_iter=None):
                """z half of in_proj(b) -> sz; optionally interleave the
                previous block's out_proj groups from ho_iter."""
                u_t = ut_store.pop(b)
                sz = pz.tile([128, ET, BLK], BF16)
                sz_store[b] = sz
                for mt in range(ET):
                    if ho_iter is not None:
                        nxt = next(ho_iter, None)
                        if nxt is not None:
                            nxt()
                    pmm = ps_mm.tile([128, BLK], FP32, tag="mm")
                    for kt in range(KT_D):
                        nc.tensor.matmul(
                            pmm[:], w_in[:, kt, mt * 128:(mt + 1) * 128],
                            u_t[:, kt, :], start=(kt == 0), stop=(kt == KT_D - 1))
                    # silu(z) = z * (0.5*tanh(z/2) + 0.5)
                    th = psm.tile([128, BLK], BF16, tag="th")
                    nc.scalar.activation(th[:], pmm[:], AF.Tanh, scale=0.5)
                    zb = psm.tile([128, BLK], BF16, tag="zb")
                    nc.scalar.copy(zb[:], pmm[:])
                    nc.vector.tensor_scalar(th[:], th[:], 0.5, 0.5,
                                            op0=AL.mult, op1=AL.add)
                    nc.gpsimd.tensor_mul(sz[:, mt, :], th[:], zb[:])

            def emit_conv(b):
                # causal depthwise conv (+bias): 4 DVE tensor-scalar products
                # (4x perf mode) + 3 Pool tensor-tensor adds, then tanh-silu
                cv = pxc.tile([128, CT, BLK], BF16)
                cv_store[b] = cv
                for ct in range(CT):
                    cvs = cv[:, ct, :]
                    cvt = pcv1.tile([128, 3, BLK], BF16, tag="cvt")
                    nc.vector.tensor_scalar(cvs, xbc[:, ct, 1:1 + BLK],
                                            cw[:, ct, 0:1], cb[:, ct:ct + 1],
                                            op0=AL.mult, op1=AL.add)
                    for k in range(1, DCONV):
                        nc.vector.tensor_scalar(cvt[:, k - 1, :],
                                                xbc[:, ct, 1 + k:1 + k + BLK],
                                                cw[:, ct, k:k + 1], None,
                                                op0=AL.mult)
                    nc.gpsimd.tensor_add(cvt[:, 0, :], cvt[:, 0, :],
                                         cvt[:, 1, :])
                    nc.gpsimd.tensor_add(cvs, cvs, cvt[:, 2, :])
                    nc.gpsimd.tensor_add(cvs, cvs, cvt[:, 0, :])
                for cp in ((0, 4), (4, 4), (8, 4), (12, 4), (16, 1)):
                    c0, n_in = cp
                    w = n_in * BLK
                    cvp = cv[:, c0:c0 + n_in, :]
                    th2 = pcm.tile([128, 4 * BLK], BF16, tag="th2")
                    nc.scalar.activation(th2[:, 0:w], cvp, AF.Tanh, scale=0.5)
                    nc.vector.tensor_scalar(th2[:, 0:w], th2[:, 0:w], 0.5, 0.5,
                                            op0=AL.mult, op1=AL.add)
                    nc.gpsimd.tensor_tensor(cvp, th2[:, 0:w], cvp, op=AL.mult)
                # cv[:, 0:16] = x (silu'd), cv[:, 16] = B (parts 0:64) | C
                ctc = psm.tile([NST, BLK], BF16, tag="ctc")
                ctc_store[b] = ctc
                nc.sync.dma_start(ctc[:], cv[NST:128, 16, :])

            sz_store = {}
            cv_store = {}
            ctc_store = {}

            prologue = prep_inproj_xbc(0)
            for fn in prologue:
                fn()
            emit_conv(0)
            emit_z(0)

            for b in range(NBLK):
                tsl = slice(b * BLK, (b + 1) * BLK)
                cv = cv_store.pop(b)
                ctc = ctc_store.pop(b)
                steal = iter(prep_inproj_xbc(b + 1) if b + 1 < NBLK else [])

                g_sb = pg.tile([128, ET, BLK], BF16)

                for qi in range(QPB):
                    qsl = slice(qi * Q, (qi + 1) * Q)
                    gsl = slice(b * BLK + qi * Q, b * BLK + (qi + 1) * Q)
                    cidx = b * QPB + qi
                    cum = pcm.tile([H, Q], FP32, tag="cum")
                    nc.vector.tensor_tensor_scan(cum[:], dtf[:, gsl], zeros32[:],
                                                 0.0, op0=AL.add, op1=AL.add)
                    ptr = ps_tr.tile([Q, H], FP32, tag="tr")
                    nc.tensor.transpose(ptr[:], cum[:], identf[0:H, 0:H])
                    ctall = pcm.tile([Q, H], FP32, tag="ctall")
                    nc.scalar.copy(ctall[:], ptr[:])
                    # exp bias per head: ln(dt_s) - cum_s  (dt folded into lt)
                    negcl = pcm.tile([Q, H], FP32, tag="negcl")
                    nc.vector.tensor_sub(negcl[:],
                                         lndttok[:, cidx * H:(cidx + 1) * H],
                                         ctall[:])

                    ptb = ps_tr.tile([Q, NST], BF16, tag="tr")
                    nc.tensor.transpose(ptb[:], cv[0:NST, 16, qsl],
                                        identb[0:NST, 0:NST])
                    btok = pcm.tile([Q, NST], BF16, tag="btok")
                    nc.scalar.copy(btok[:], ptb[:])

                    g0 = ps_tr.tile([Q, Q], FP32, tag="tr")
                    nc.tensor.matmul(g0[:], cv[0:NST, 16, qsl], ctc[:, qsl])
                    g0sb = pcm.tile([Q, Q], BF16, tag="g0sb")
                    nc.scalar.copy(g0sb[:], g0[:])

                    xtok = pxt.tile([Q, E], BF16, tag="xtok")
                    for f4 in range(ET // 4):
                        ptx = ps_tr.tile([Q, 4 * 128], BF16, tag="tr")
                        for j in range(4):
                            nc.tensor.transpose(
                                ptx[:, j * 128:(j + 1) * 128],
                                cv[:, 4 * f4 + j, qsl], identb[:])
                        nc.vector.tensor_copy(
                            xtok[:, f4 * 512:(f4 + 1) * 512], ptx[:])

                    for _ in range(2):
                        fn = next(steal, None)
                        if fn is not None:
                            fn()
                    for hg in range(H // 8):
                        h0 = hg * 8
                        bcq8 = pb2.tile([Q, 8 * Q], FP32, tag="bcq")
                        for hh in range(2):
                            stg = psc.tile([1, 4 * Q], FP32, tag="stg")
                            nc.sync.dma_start(
                                stg[:], cum[h0 + 4 * hh:h0 + 4 * hh + 4, :])
                            nc.gpsimd.partition_broadcast(
                                bcq8[:, 4 * hh * Q:4 * (hh + 1) * Q], stg[:])
                        bce8 = pb1.tile([Q, 8 * Q], FP32, tag="bce")
                        nc.scalar.activation(bce8[:], bcq8[:], AF.Exp)
                        # mask AFTER bce8 snapshot (WAR dep keeps order)
                        nc.gpsimd.tensor_add(bcq8[:], bcq8[:], mask8[:])
                        for kp in range(4):
                            ftp = (h0 + 2 * kp) // 2
                            yp2 = ps_yp.tile([128, Q], FP32, tag="yp")
                            for k2 in range(2):
                                k = 2 * kp + k2
                                h = h0 + k
                                ro = k2 * 64
                                csl = slice(h * HD, (h + 1) * HD)
                                ksl = slice(k * Q, (k + 1) * Q)
                                lt = psc.tile([Q, Q], FP32, tag="lt")
                                nc.scalar.activation(lt[:], bcq8[:, ksl],
                                                     AF.Exp,
                                                     bias=negcl[:, h:h + 1])
                                mt_t = psc.tile([Q, Q], BF16, tag="mt")
                                nc.gpsimd.tensor_mul(mt_t[:], g0sb[:], lt[:])
                                cpos = psc.tile([NST, Q], BF16, tag="cpos")
                                nc.gpsimd.tensor_mul(cpos[:], ctc[:, qsl],
                                                     bce8[0:NST, ksl])
                                bh = psc.tile([Q, NST], BF16, tag="bh")
                                nc.vector.tensor_scalar(
                                    bh[:], btok[:],
                                    lt[:, Q - 1:Q],
                                    None, op0=AL.mult)
                                nc.tensor.matmul(yp2[ro:ro + 64, :],
                                                 xtok[:, csl], mt_t[:],
                                                 start=True, stop=False)
                                nc.tensor.matmul(yp2[ro:ro + 64, :],
                                                 S[:, csl], cpos[:],
                                                 start=False, stop=True)
                                sp = ps_sp.tile([NST, HD], FP32, tag="sp")
                                nc.tensor.matmul(sp[:], bh[:], xtok[:, csl])
                                nc.vector.scalar_tensor_tensor(
                                    S[:, csl], S[:, csl],
                                    bce8[0:NST, (k + 1) * Q - 1:(k + 1) * Q],
                                    sp[:], op0=AL.mult, op1=AL.add)
                            nc.vector.scalar_tensor_tensor(
                                g_sb[:, ftp, qsl], cv[:, ftp, qsl],
                                dxt[:, ftp:ftp + 1], yp2[:],
                                op0=AL.mult, op1=AL.add)
                        # weave a couple of next-block in_proj pieces into
                        # the scan so PE/DVE gaps get filled
                        for _ in range(2):
                            fn = next(steal, None)
                            if fn is not None:
                                fn()
                for fn in steal:
                    fn()

                # ---- gating, sum of squares (scale deferred)
                sz = sz_store.pop(b)
                for gq in range(4):
                    nc.gpsimd.tensor_tensor(g_sb[:, 4 * gq:4 * gq + 4, :],
                                            g_sb[:, 4 * gq:4 * gq + 4, :],
                                            sz[:, 4 * gq:4 * gq + 4, :],
                                            op=AL.mult)
                ssq = ps_mm.tile([1, BLK], FP32, tag="mm")
                for ft in range(ET):
                    g2 = psm.tile([128, BLK], BF16, tag="th")
                    nc.vector.tensor_tensor(g2[:], g_sb[:, ft, :],
                                            g_sb[:, ft, :], op=AL.mult)
                    nc.tensor.matmul(ssq[:], onesb[:], g2[:],
                                     start=(ft == 0), stop=(ft == ET - 1))
                nc.vector.tensor_scalar(mall[0:1, tsl], ssq[:], 1.0 / E, EPS,
                                        op0=AL.mult, op1=AL.add)

                # conv of the next block runs on DVE/Pool while out_proj
                # owns the TensorEngine
                if b + 1 < NBLK:
                    emit_conv(b + 1)

                def mk_ho(mt):
                    def go():
                        ho = ps_mm.tile([128, BLK], FP32, tag="mm")
                        for kt in range(ET):
                            nc.tensor.matmul(
                                ho[:], w_out[:, kt, mt * 128:(mt + 1) * 128],
                                g_sb[:, kt, :],
                                start=(kt == 0), stop=(kt == ET - 1))
                        hob = psm.tile([128, BLK], BF16, tag="hob")
                        nc.vector.tensor_copy(hob[:], ho[:])
                        nc.sync.dma_start(ho_d[:, mt, tsl], hob[:])
                    return go
                ho_steps = iter([mk_ho(mt) for mt in range(ET // 2)])
                if b + 1 < NBLK:
                    emit_z(b + 1, ho_iter=ho_steps)
                for fn in ho_steps:
                    fn()

            # ---- layer epilogue: rmsnorm scale + selu (+ spill / pooling)
            nc.scalar.activation(mall[:], mall[:], AF.Ln)
            nc.scalar.activation(mall[:], mall[:], AF.Exp, scale=-0.5)
            for b in range(NBLK):
                tsl = slice(b * BLK, (b + 1) * BLK)
                scb = psm.tile([128, BLK], BF16, tag="scb")
                nc.gpsimd.partition_broadcast(scb[:], mall[0:1, tsl])
                if layer == 0:
                    u2blk = pio.tile([128, KT_D, BLK], BF16, tag="ut")
                for mt in range(ET // 2):
                    hot = pio.tile([128, BLK], BF16, tag="hot")
                    nc.sync.dma_start(hot[:], ho_d[:, mt, tsl])
                    nc.gpsimd.tensor_mul(hot[:], hot[:], scb[:])
                    rl = psm.tile([128, BLK], BF16, tag="rl")
                    nc.scalar.activation(rl[:], hot[:], AF.Relu, scale=SELU_L)
                    ex = psm.tile([128, BLK], BF16, tag="ex")
                    nc.scalar.activation(ex[:], hot[:], AF.Exp, bias=lnla_t[:])
                    nc.vector.tensor_scalar(ex[:], ex[:], SELU_LA, SELU_LA,
                                            op0=AL.min, op1=AL.subtract)
                    if layer == 0:
                        nc.gpsimd.tensor_tensor(u2blk[:, mt, :], rl[:], ex[:],
                                                op=AL.add)
                    else:
                        nc.gpsimd.tensor_tensor(rl[:], rl[:], ex[:], op=AL.add)
                        red = psm.tile([128, 1], FP32, tag="red")
                        nc.vector.tensor_reduce(red[:], rl[:],
                                                axis=mybir.AxisListType.X,
                                                op=AL.add)
                        nc.vector.tensor_add(pacc[:, mt:mt + 1],
                                             pacc[:, mt:mt + 1], red[:])
                if layer == 0:
                    nc.sync.dma_start(u2[:, :, tsl], u2blk[:])
                    pmmd = ps_mm.tile([H, BLK], FP32, tag="mm")
                    for kt in range(KT_D):
                        nc.tensor.matmul(pmmd[:], wdt2[:, kt, :],
                                         u2blk[:, kt, :], start=(kt == 0),
                                         stop=(kt == KT_D - 1))
                    nc.scalar.activation(dtf[:, tsl], pmmd[:], AF.Exp,
                                         bias=dtb2c[:])

        pooled = psm.tile([128, KT_D], FP32, tag="pooled")
        nc.vector.tensor_scalar(pooled[:], pacc[:, 0:KT_D], 1.0 / L, None,
                                op0=AL.mult)
        ph = ps_sp.tile([1, 10], FP32, tag="sp")
        for kt in range(KT_D):
            nc.tensor.matmul(ph[:], pooled[:, kt:kt + 1], whead[:, kt, :],
                             start=(kt == 0), stop=(kt == KT_D - 1))
        ot = psm.tile([1, 10], FP32, tag="ot")
        nc.vector.tensor_add(ot[:], ph[:], bcat[:])
        nc.sync.dma_start(out_d[:], ot[:])

    nc.compile()
    return nc


def _host_inputs(inputs):
    m = {}
    m.update(_prep_layer(inputs, "1"))
    m.update(_prep_layer(inputs, "2"))
    j = np.arange(128)
    mneg = np.where(j[None, :] >= j[:, None], 0.0, -1e30)
    m["mask8"] = _bf(np.tile(mneg, (1, 8)))
    m["identb"] = _bf(np.eye(128))
    m["identf"] = _f32(np.eye(128))
    m["onesb"] = _bf(np.ones((128, 1)))
    wcat = np.concatenate([np.asarray(inputs["w_emo"], np.float32),
                           np.asarray(inputs["w_sen"], np.float32)], 0)
    m["whead"] = _f32(wcat.T.reshape(KT_D, 128, 10).transpose(1, 0, 2))
    m["bcat"] = _f32(np.concatenate([inputs["b_emo"], inputs["b_sen"]])
                     .reshape(1, 10))
    return m


def kernel(**inputs) -> np.ndarray:
    if "nc" not in _CACHE:
        _CACHE["nc"] = _build()
    nc = _CACHE["nc"]

    x = np.asarray(inputs["x"], np.float32)
    shared = _host_inputs(inputs)
    in_maps = []
    for s in range(NCORE):
        m = dict(shared)
        xts = x[s].T.reshape(KT_D, 128, L).transpose(1, 0, 2)
        m["xt"] = _bf(xts)
        in_maps.append(m)

    res = run_bass_kernel_spmd(nc, in_maps, core_ids=list(range(NCORE)))
    out = np.concatenate([r["out"] for r in res.results], 0)
    return out.astype(np.float32)
